# revision 33
# baseline (speedup 1.0000x reference)
"""Dual-stream linear-attention transformer kernel (per-core), v3.

v3: per-m interleaved emission across the body/limb pair (ACT has no
exec-queue lookahead, so fine-grained alternation is what keeps it fed),
LN stats pairs share one PSUM tile (rows 0/1), fp16 operands throughout.
See v2 notes below.

  - fp16 matmul operands + activations + residual DRAM (fp32 PSUM/stats).
  - q/k/v low-rank projections premultiplied on host to single [E,E]
    mats; k/v produced directly token-major by using x as lhsT.
  - depthwise conv on PE via host-built diagonal tap matrices (BN scale
    folded in) against halo-padded h tiles.
  - attention denominator scaled by 1/64 so fp16 reciprocals stay in
    the normal range (1/64 folded into the bd kv blocks).

Layouts:
  - layout 1: [E, N] feature-major; SBUF tiles [128, KE, C].
  - layout 2 (k/v only): [tok, E] token-major.
  - Residuals in internal DRAM as fp16 [E, N] -> p k n.
"""

from dataclasses import dataclass
from contextlib import ExitStack

import numpy as np

import concourse.bass as bass
import concourse.mybir as mybir
import concourse.tile as tile

F32 = mybir.dt.float32
F16 = mybir.dt.float16
AF = mybir.ActivationFunctionType
ALU = mybir.AluOpType

LN_EPS = 1e-5
BN_EPS = 1e-5
DEN_SCALE = 1.0 / 64.0


@dataclass
class Cfg:
    N: int = 2048
    E: int = 512
    R: int = 256
    X: int = 1024
    H: int = 8
    L: int = 3
    OUT: int = 15
    C: int = 512

    @property
    def KE(self):
        return self.E // 128

    @property
    def KX(self):
        return self.X // 128

    @property
    def NC(self):
        return self.N // self.C

    @property
    def NTT(self):
        return self.C // 128


def host_constants(cfg, inputs):
    """Precompute fp16 weights / fused constants on host."""
    c = cfg
    f = lambda a: np.ascontiguousarray(a, dtype=np.float32)
    h = lambda a: np.ascontiguousarray(a, dtype=np.float16)
    dw, uw = f(inputs["dw"]), f(inputs["uw"])
    qkvw = np.einsum("latir,latrj->latij", dw, uw)
    out = {
        "body_feats": h(inputs["body_feats"]),
        "limb_feats": h(inputs["limb_feats"]),
        "qkvw": h(qkvw),
        "ub": f(inputs["ub"]),
        "ub16": h(inputs["ub"]),
        "ow": h(inputs["ow"]),
        "ob": f(inputs["ob"]),
        "w1": h(inputs["w1"]),
        "b1": f(inputs["b1"]),
        "w2": h(inputs["w2"]),
        "b2": f(inputs["b2"]),
        "lng": f(inputs["lng"]),
        "lnb": f(inputs["lnb"]),
        "gw1": h(inputs["gw1"]),
        "gb1": f(inputs["gb1"]),
        "gwd": h(f(inputs["gw2"])[:, :, 0] - f(inputs["gw2"])[:, :, 1]),
        "gb2d": f(f(inputs["gb2"])[:, 0:1] - f(inputs["gb2"])[:, 1:2]),
        "fw1": h(inputs["fw1"]),
        "fb1": f(inputs["fb1"]),
        "fw2": h(inputs["fw2"]),
        "fb2": f(inputs["fb2"]),
        "flng": f(inputs["flng"]),
        "flnb": f(inputs["flnb"]),
        "rw1": h(inputs["rw1"]),
        "rb1": f(inputs["rb1"]),
        "rw2": h(inputs["rw2"]),
        "rb2": f(inputs["rb2"]),
    }
    rw3 = np.zeros((c.E // 8, 16), np.float16)
    rw3[:, : c.OUT] = f(inputs["rw3"])
    out["rw3p"] = rw3
    rb3 = np.zeros((1, 16), np.float16)
    rb3[0, : c.OUT] = f(inputs["rb3"])
    out["rb3p"] = rb3
    rsq = 1.0 / np.sqrt(1.0 + BN_EPS)
    A = f(inputs["bng"]) * rsq
    cw, cb = f(inputs["cw"]), f(inputs["cb"])
    taps = np.zeros((c.L, 2, c.KX, 3, 128, 128), np.float16)
    idx = np.arange(128)
    for t in range(3):
        wA = (cw[:, :, :, t] * A).reshape(c.L, 2, c.KX, 128)
        taps[:, :, :, t, idx, idx] = wA.astype(np.float16)
    out["taps"] = taps
    out["convB"] = f(cb * A + f(inputs["bnb"]))
    out["ident"] = np.eye(128, dtype=np.float16)
    out["ones16"] = np.ones((128, 128), np.float16)
    E, H = c.E, c.H
    dh = E // H
    hmask = np.zeros((E, H), np.float16)
    for ff in range(E):
        hmask[ff, ff // dh] = 1.0
    out["hmask"] = hmask
    out["cmask"] = np.ascontiguousarray(hmask.T)
    return out


PHASES = []


def build(nc, cfg):
    c = cfg
    E, X, H, N, C, L = c.E, c.X, c.H, c.N, c.C, c.L
    KE, KX, NC, NTT = c.KE, c.KX, c.NC, c.NTT
    E4, E2, E8 = E // 4, E // 2, E // 8

    din = {}

    def inp(name, shape, dt):
        din[name] = nc.dram_tensor(name, list(shape), dt, kind="ExternalInput")
        return din[name].ap()

    body_feats = inp("body_feats", (N, E), F16)
    limb_feats = inp("limb_feats", (N, E), F16)
    qkvw = inp("qkvw", (L, 4, 3, E, E), F16)
    ub = inp("ub", (L, 4, 3, E), F32)
    ub16 = inp("ub16", (L, 4, 3, E), F16)
    ow = inp("ow", (L, 4, E, E), F16)
    ob = inp("ob", (L, 4, E), F32)
    w1 = inp("w1", (L, 2, E, X), F16)
    b1 = inp("b1", (L, 2, X), F32)
    w2 = inp("w2", (L, 2, X, E), F16)
    b2 = inp("b2", (L, 2, E), F32)
    taps = inp("taps", (L, 2, KX, 3, 128, 128), F16)
    convB = inp("convB", (L, 2, X), F32)
    lng = inp("lng", (L, 5, E), F32)
    lnb = inp("lnb", (L, 5, E), F32)
    gw1 = inp("gw1", (L, 2 * E, E4), F16)
    gb1 = inp("gb1", (L, E4), F32)
    gwd = inp("gwd", (L, E4), F16)
    gb2d = inp("gb2d", (L, 1), F32)
    fw1 = inp("fw1", (2 * E, E2), F16)
    fb1 = inp("fb1", (E2,), F32)
    fw2 = inp("fw2", (E2, E), F16)
    fb2 = inp("fb2", (E,), F32)
    flng = inp("flng", (E,), F32)
    flnb = inp("flnb", (E,), F32)
    rw1 = inp("rw1", (E, E4), F16)
    rb1 = inp("rb1", (E4,), F32)
    rw2 = inp("rw2", (E4, E8), F16)
    rb2 = inp("rb2", (E8,), F32)
    rw3p = inp("rw3p", (E8, 16), F16)
    rb3p = inp("rb3p", (1, 16), F16)
    ident_in = inp("ident", (128, 128), F16)
    ones_in = inp("ones16", (128, 128), F16)
    hmask_in = inp("hmask", (E, H), F16)
    cmask_in = inp("cmask", (H, E), F16)

    out_dram = nc.dram_tensor("out", [N, c.OUT], F32, kind="ExternalOutput")

    def idram(name):
        return [nc.dram_tensor(f"{name}_c{ci}", [E, C], F16).ap().rearrange(
            "(k p) n -> p k n", p=128) for ci in range(NC)]

    rs = {}
    for s in ("b", "l"):
        rs[s, 0] = idram(f"r{s}0")
        for l in range(L):
            for st in (1, 2, 3):
                rs[s, (l, st)] = idram(f"r{s}_{l}_{st}")

    lowp = nc.allow_low_precision("fp16 activations within rel-err budget")

    with tile.TileContext(nc) as tc, ExitStack() as ctx, lowp:
        p_ = ctx.enter_context
        cst = p_(tc.tile_pool(name="cst", bufs=1))
        wq = p_(tc.tile_pool(name="wq", bufs=2))       # big weights
        wcol = p_(tc.tile_pool(name="wcol", bufs=8))   # bias cols
        wrow = p_(tc.tile_pool(name="wrow", bufs=2))   # bias rows
        pa = p_(tc.tile_pool(name="pa", bufs=10))      # 4KB fp16 act tiles
        pb = p_(tc.tile_pool(name="pb", bufs=4))       # ffn h tiles
        pc = p_(tc.tile_pool(name="pc", bufs=9))       # small tiles
        pat = p_(tc.tile_pool(name="pat", bufs=2))     # attn persistents
        ps = p_(tc.tile_pool(name="ps", bufs=5, space="PSUM"))
        pskv = p_(tc.tile_pool(name="pskv", bufs=1, space="PSUM"))
        psst = p_(tc.tile_pool(name="psst", bufs=2, space="PSUM"))

        v, sc, gp = nc.vector, nc.scalar, nc.gpsimd

        def mm(out, lhsT, rhs, start, stop):
            nc.tensor.matmul(out, lhsT, rhs, start=start, stop=stop)

        # ---- constants ----
        ident_t = cst.tile([128, 128], F16, tag="ident")
        nc.sync.dma_start(out=ident_t, in_=ident_in)
        ones_t = cst.tile([128, 128], F16, tag="ones")
        nc.sync.dma_start(out=ones_t, in_=ones_in)
        hmask_t = cst.tile([128, KE, H], F16, tag="hmask")
        nc.sync.dma_start(out=hmask_t,
                          in_=hmask_in.rearrange("(k p) h -> p k h", p=128))
        cmask_t = cst.tile([H, KE, 128], F16, tag="cmask")
        nc.sync.dma_start(out=cmask_t,
                          in_=cmask_in.rearrange("h (k p) -> h k p", p=128))
        ONES_COL = ones_t[:, 0:1]
        ONES_ROW = ones_t[0:1, :]
        eps_den = cst.tile([8, 1], F32, tag="epsd")
        v.memset(eps_den, 1e-6 * DEN_SCALE)
        eps_ln = cst.tile([1, 1], F32, tag="epsl")
        v.memset(eps_ln, LN_EPS)

        def col_tile(src_ap, m, tag="col"):
            t = wcol.tile([128, m], F32, tag=tag)
            nc.sync.dma_start(out=t, in_=src_ap.rearrange("(m p) -> p m", p=128))
            return t

        def ln_pair(jobs, relu=False, apply_dve=False):
            """LayerNorm over features, pair-interleaved.
            jobs: list of (xs, g_col, b_col, outt); xs/outt fp16
            [128, KE, C]; outt doubles as x^2 scratch."""
            for xs, _, _, outt in jobs:
                v.tensor_tensor(out=outt, in0=xs, in1=xs, op=ALU.mult)
            pst = []
            for xs, _, _, outt in jobs:
                pp_s = psst.tile([8, C], F32, tag="st", name="lnst")
                pp_q = psst.tile([8, C], F32, tag="st", name="lnsq")
                for m in range(KE):
                    mm(pp_s[0:1, :], ONES_COL, xs[:, m, :], start=(m == 0),
                       stop=(m == KE - 1))
                for m in range(KE):
                    mm(pp_q[0:1, :], ONES_COL, outt[:, m, :], start=(m == 0),
                       stop=(m == KE - 1))
                pst.append((pp_s, pp_q))
            stts = []
            for (xs, _, _, _), (pp_s, pp_q) in zip(jobs, pst):
                arow = pc.tile([1, C], F32, tag="s2", bufs=4, name="arow")
                brow = pc.tile([1, C], F32, tag="s2", bufs=4, name="brow")
                sc.activation(arow, pp_s[0:1, :], AF.Copy, scale=1.0 / E)
                sc.activation(brow, pp_q[0:1, :], AF.Copy, scale=1.0 / E)
                trow2 = pc.tile([1, C], F32, tag="s2", bufs=4)
                sc.activation(trow2, arow, AF.Square)
                v.tensor_tensor(out=brow, in0=brow, in1=trow2,
                                op=ALU.subtract)
                sc.activation(brow, brow, AF.Sqrt, bias=eps_ln[0:1, 0:1])
                stt = pc.tile([1, 2, C], F16, tag="s2", bufs=4)
                v.reciprocal(out=stt[:, 0, :], in_=brow)
                v.tensor_tensor(out=stt[:, 1, :], in0=arow, in1=stt[:, 0, :],
                                op=ALU.mult)
                stts.append(stt)
            sbs = []
            for stt in stts:
                bc_s = ps.tile([128, C], F32, tag="mm")
                mm(bc_s, ones_t[0:1, :], stt[0:1, 0, :], start=True, stop=True)
                bc_t = ps.tile([128, C], F32, tag="mm")
                mm(bc_t, ones_t[0:1, :], stt[0:1, 1, :], start=True, stop=True)
                sb = pc.tile([128, 2, C], F16, tag="c2", bufs=2)
                sc.activation(sb[:, 0, :], bc_s, AF.Copy)
                sc.activation(sb[:, 1, :], bc_t, AF.Copy)
                sbs.append(sb)
            fn = AF.Relu if relu else AF.Identity
            for m in range(KE):
                for (xs, g_col, b_col, outt), sb in zip(jobs, sbs):
                    t1 = pc.tile([128, C], F16, tag="c1")
                    v.tensor_tensor(out=t1, in0=xs[:, m, :], in1=sb[:, 0, :],
                                    op=ALU.mult)
                    v.tensor_tensor(out=t1, in0=t1, in1=sb[:, 1, :],
                                    op=ALU.subtract)
                    if apply_dve:
                        v.tensor_scalar(out=t1, in0=t1,
                                        scalar1=g_col[:, m:m + 1],
                                        scalar2=b_col[:, m:m + 1],
                                        op0=ALU.mult, op1=ALU.add)
                        if relu:
                            v.tensor_scalar_max(outt[:, m, :], t1, 0.0)
                        else:
                            v.tensor_copy(outt[:, m, :], t1)
                    else:
                        sc.activation(outt[:, m, :], t1, fn,
                                      bias=b_col[:, m:m + 1],
                                      scale=g_col[:, m:m + 1])

        def load_x_chunk(dram_l1, ci, tag="a4"):
            xt = pa.tile([128, KE, C], F16, tag=tag)
            nc.sync.dma_start(out=xt, in_=dram_l1[ci])
            return xt

        def store_chunk(dram_l1, ci, t):
            gp.dma_start(out=dram_l1[ci], in_=t)

        # ---- entry transpose (interleaved) ----
        def entry_tile(x_ap, dst, ttk):
            x2 = pa.tile([128, E], F16, tag="a4")
            nc.sync.dma_start(out=x2, in_=x_ap[ttk * 128:(ttk + 1) * 128, :])
            xt = pa.tile([128, KE, 128], F16, tag="a4")
            for f in range(KE):
                pt = ps.tile([128, 128], F16, tag="mm")
                nc.tensor.transpose(pt, x2[:, f * 128:(f + 1) * 128], ident_t)
                if f % 2 == 0:
                    sc.activation(xt[:, f, :], pt, AF.Copy)
                else:
                    v.tensor_copy(xt[:, f, :], pt)
            tl = ttk % NTT
            nc.sync.dma_start(out=dst[ttk // NTT][:, :, tl * 128:(tl + 1) * 128],
                              in_=xt)

        PHASES.append(("entry", len(nc.inst_map)))
        for ttk in range(N // 128):
            entry_tile(body_feats, rs["b", 0], ttk)
            entry_tile(limb_feats, rs["l", 0], ttk)

        # ---- linear attention (pairs) ----
        def attn_setup(l, a):
            st = {}
            kvw = wq.tile([128, KE, 2, E], F16, tag="kvw")
            for t3 in (1, 2):
                nc.sync.dma_start(
                    out=kvw[:, :, t3 - 1, :],
                    in_=qkvw[l, a, t3].rearrange("(k p) e -> p k e", p=128))
            qw = wq.tile([128, KE, E], F16, tag="qw")
            nc.sync.dma_start(
                out=qw, in_=qkvw[l, a, 0].rearrange("(k p) e -> p k e", p=128))
            owt = wq.tile([128, KE, E], F16, tag="ow")
            nc.sync.dma_start(
                out=owt, in_=ow[l, a].rearrange("(k p) e -> p k e", p=128))
            st["kvw"], st["qw"], st["owt"] = kvw, qw, owt
            st["ubq_col"] = col_tile(ub[l, a, 0], KE)
            ubkv = wrow.tile([1, 2, E], F16, tag="row")
            nc.sync.dma_start(out=ubkv[:, 0, :], in_=ub16[l, a, 1][None, :])
            nc.sync.dma_start(out=ubkv[:, 1, :], in_=ub16[l, a, 2][None, :])
            st["ubkv"] = ubkv
            st["ob_col"] = col_tile(ob[l, a], KE)
            st["kv_acc"] = pat.tile([128, 4, 258], F32, tag="kva",
                                    name="kv_acc")
            return st

        def alpha_pair_step(sts, srcs, ci):
            xts = [load_x_chunk(src, ci) for src in srcs]
            work = []
            for st, xt in zip(sts, xts):
                k2f = pa.tile([128, NTT, E], F16, tag="a4", name="k2f")
                v2x = pa.tile([128, NTT, 2, 258], F16, tag="a4", name="v2x")
                v.memset(v2x[:, :, :, 256:258], 1.0)
                work.append((st, xt, k2f, v2x))
            for tt in range(NTT):
                for st, xt, k2f, v2x in work:
                    kvw = st["kvw"]
                    pk = ps.tile([128, E], F32, tag="mm")
                    pv = ps.tile([128, E], F32, tag="mm")
                    for k in range(KE):
                        lx = xt[:, k, tt * 128:(tt + 1) * 128]
                        mm(pk, lx, kvw[:, k, 0, :], start=(k == 0), stop=False)
                        mm(pv, lx, kvw[:, k, 1, :], start=(k == 0), stop=False)
                    mm(pk, ONES_ROW, st["ubkv"][:, 0, :], start=False,
                       stop=True)
                    mm(pv, ONES_ROW, st["ubkv"][:, 1, :], start=False,
                       stop=True)
                    ee = pc.tile([128, E], F16, tag="c1")
                    rr = pc.tile([128, E], F16, tag="c1")
                    sc.activation(ee, pk, AF.Exp)
                    v.tensor_scalar_max(rr, pk, 0.0)
                    v.tensor_scalar_min(ee, ee, 1.0)
                    v.tensor_tensor(out=k2f[:, tt, :], in0=ee, in1=rr,
                                    op=ALU.add)
                    sc.activation(v2x[:, tt, 0, 0:256], pv[:, 0:256], AF.Copy)
                    sc.activation(v2x[:, tt, 1, 0:256], pv[:, 256:512],
                                  AF.Copy)
            for p in range(4):
                for st, xt, k2f, v2x in work:
                    pkv = pskv.tile([128, 258], F32, tag="kv")
                    for tt in range(NTT):
                        mm(pkv, k2f[:, tt, p * 128:(p + 1) * 128],
                           v2x[:, tt, p // 2, :],
                           start=(tt == 0), stop=(tt == NTT - 1))
                    kv_acc = st["kv_acc"]
                    if ci == 0:
                        sc.activation(kv_acc[:, p, :], pkv, AF.Copy)
                    else:
                        v.tensor_tensor(out=kv_acc[:, p, :],
                                        in0=kv_acc[:, p, :], in1=pkv,
                                        op=ALU.add)

        def alpha_fin(st):
            kv_acc = st["kv_acc"]
            bd = pat.tile([128, KE, 128], F16, tag="bd")
            v.memset(bd, 0.0)
            for p in range(4):
                h0c = (2 * p % 4) * 64
                h1c = ((2 * p + 1) % 4) * 64
                v.tensor_scalar_mul(bd[0:64, p, 0:64],
                                    kv_acc[0:64, p, h0c:h0c + 64], DEN_SCALE)
                v.tensor_scalar_mul(bd[64:128, p, 64:128],
                                    kv_acc[64:128, p, h1c:h1c + 64], DEN_SCALE)
            kmm = pat.tile([128, KE, H], F16, tag="km")
            for k in range(KE):
                v.tensor_scalar(out=kmm[:, k, :], in0=hmask_t[:, k, :],
                                scalar1=kv_acc[:, k, 256:257],
                                scalar2=DEN_SCALE, op0=ALU.mult, op1=ALU.mult)
            st["bd"], st["kmm"] = bd, kmm

        def beta_pair_step(sts, srcs, ci, tails):
            xqs = [load_x_chunk(src, ci) for src in srcs]
            qfs = [pa.tile([128, KE, C], F16, tag="a4", name="qf")
                   for _ in sts]
            pds = [psst.tile([8, C], F32, tag="st", name="pd")
                   for _ in sts]
            for m in range(KE):
                for st, xq, qf, pd in zip(sts, xqs, qfs, pds):
                    qw = st["qw"]
                    pq = ps.tile([128, C], F32, tag="mm")
                    for k in range(KE):
                        mm(pq, qw[:, k, m * 128:(m + 1) * 128],
                           xq[:, k, :], start=(k == 0), stop=(k == KE - 1))
                    ee = pc.tile([128, C], F16, tag="c1")
                    rr = pc.tile([128, C], F16, tag="c1")
                    sc.activation(ee, pq, AF.Exp,
                                  bias=st["ubq_col"][:, m:m + 1])
                    v.tensor_scalar(out=rr, in0=pq,
                                    scalar1=st["ubq_col"][:, m:m + 1],
                                    scalar2=0.0, op0=ALU.add, op1=ALU.max)
                    v.tensor_scalar_min(ee, ee, 1.0)
                    v.tensor_tensor(out=qf[:, m, :], in0=ee, in1=rr,
                                    op=ALU.add)
                    mm(pd, st["kmm"][:, m, :], qf[:, m, :], start=(m == 0),
                       stop=(m == KE - 1))
            recs = []
            for st, pd in zip(sts, pds):
                rec = pc.tile([8, C], F16, tag="s2", bufs=4)
                v.reciprocal(out=rec, in_=pd)
                recs.append(rec)
            atts = [pa.tile([128, KE, C], F16, tag="a4", name="att")
                    for _ in sts]
            for m in range(KE):
                for st, qf, att, rec in zip(sts, qfs, atts, recs):
                    pn = ps.tile([128, C], F32, tag="mm")
                    mm(pn, st["bd"][:, m, :], qf[:, m, :], start=True,
                       stop=True)
                    pr = ps.tile([128, C], F32, tag="mm")
                    mm(pr, cmask_t[:, m, :], rec, start=True, stop=True)
                    rb = pc.tile([128, C], F16, tag="c1")
                    sc.activation(rb, pr, AF.Copy)
                    v.tensor_tensor(out=att[:, m, :], in0=pn, in1=rb,
                                    op=ALU.mult)
            projs = [pa.tile([128, KE, C], F16, tag="a4", name="proj")
                     for _ in sts]
            for m in range(KE):
                for st, att, proj in zip(sts, atts, projs):
                    po = ps.tile([128, C], F32, tag="mm")
                    for k in range(KE):
                        mm(po, st["owt"][:, k, m * 128:(m + 1) * 128],
                           att[:, k, :], start=(k == 0), stop=(k == KE - 1))
                    sc.activation(proj[:, m, :], po, AF.Identity,
                                  bias=st["ob_col"][:, m:m + 1])
            tails[0](ci, projs, xqs)

        # ---- tails (pair) ----
        def make_self_tail_pair(l, dsts):
            cols = []
            for i, s in enumerate(("b", "l")):
                g_col = col_tile(lng[l, i], KE, tag="lncol")
                b_col = col_tile(lnb[l, i], KE, tag="lncol")
                cols.append((g_col, b_col))

            def tail(ci, projs, xqs):
                jobs = []
                for (g_col, b_col), proj, xq, dst in zip(cols, projs, xqs,
                                                         dsts):
                    v.tensor_tensor(out=proj, in0=proj, in1=xq, op=ALU.add)
                for (g_col, b_col), proj, xq, dst in zip(cols, projs, xqs,
                                                         dsts):
                    outt = pa.tile([128, KE, C], F16, tag="a4", name="outt")
                    jobs.append((proj, g_col, b_col, outt))
                ln_pair(jobs)
                for (j, dst) in zip(jobs, dsts):
                    store_chunk(dst, ci, j[3])

            return tail

        def make_cross_tail_pair(l, dsts):
            gw1t = wq.tile([128, 2 * KE, E4], F16, tag="gw")
            nc.sync.dma_start(out=gw1t,
                              in_=gw1[l].rearrange("(k p) g -> p k g", p=128))
            gwd_col = wcol.tile([128, 1], F16, tag="gwd")
            nc.sync.dma_start(out=gwd_col,
                              in_=gwd[l].rearrange("(m p) -> p m", p=128))
            gb1_col = col_tile(gb1[l], 1, tag="lncol")
            gb2d_t = wcol.tile([1, 1], F32, tag="gb2d")
            nc.sync.dma_start(out=gb2d_t, in_=gb2d[l][None, :])
            g_col = col_tile(lng[l, 2], KE, tag="lncol")
            b_col = col_tile(lnb[l, 2], KE, tag="lncol")

            def tail(ci, projs, xqs):
                bgts = []
                for proj, xq in zip(projs, xqs):
                    pg = ps.tile([128, C], F32, tag="mm")
                    for k in range(2 * KE):
                        rhs = xq[:, k, :] if k < KE else proj[:, k - KE, :]
                        mm(pg, gw1t[:, k, :], rhs, start=(k == 0),
                           stop=(k == 2 * KE - 1))
                    g1f = pc.tile([128, C], F16, tag="c1")
                    sc.activation(g1f, pg, AF.Relu, bias=gb1_col[:, 0:1])
                    g1t = pc.tile([128, C], F16, tag="c1")
                    v.tensor_scalar_min(g1t, g1f, 6.0)
                    pg2 = psst.tile([8, C], F32, tag="st", name="pg2")
                    mm(pg2[0:1, :], gwd_col, g1t, start=True, stop=True)
                    bgf = pc.tile([1, C], F16, tag="s2", bufs=4)
                    sc.activation(bgf, pg2[0:1, :], AF.Sigmoid,
                                  bias=gb2d_t[0:1, 0:1])
                    pbg = ps.tile([128, C], F32, tag="mm")
                    mm(pbg, ones_t[0:1, :], bgf, start=True, stop=True)
                    bgt = pc.tile([128, C], F16, tag="c1")
                    sc.activation(bgt, pbg, AF.Copy)
                    bgts.append(bgt)
                jobs = []
                for proj, xq, bgt in zip(projs, xqs, bgts):
                    mt = pa.tile([128, KE, C], F16, tag="a4", name="mt")
                    v.tensor_tensor(out=mt, in0=xq, in1=proj, op=ALU.subtract)
                    for m in range(KE):
                        v.tensor_tensor(out=mt[:, m, :], in0=mt[:, m, :],
                                        in1=bgt, op=ALU.mult)
                    v.tensor_tensor(out=mt, in0=mt, in1=proj, op=ALU.add)
                    outt = pa.tile([128, KE, C], F16, tag="a4", name="outt")
                    jobs.append((mt, g_col, b_col, outt))
                ln_pair(jobs)
                for (j, dst) in zip(jobs, dsts):
                    store_chunk(dst, ci, j[3])

            return tail

        # ---- FFN pair ----
        def ffn_setup(l, s):
            si = 0 if s == "b" else 1
            st = {}
            w1t = wq.tile([128, KE, X], F16, tag="w1")
            nc.sync.dma_start(
                out=w1t, in_=w1[l, si].rearrange("(k p) x -> p k x", p=128))
            w2t = wq.tile([128, KX, E], F16, tag="w2")
            nc.sync.dma_start(
                out=w2t, in_=w2[l, si].rearrange("(k p) e -> p k e", p=128))
            tapt = wq.tile([128, KX, 3, 128], F16, tag="tp" + s, bufs=1)
            nc.sync.dma_start(out=tapt,
                              in_=taps[l, si].rearrange("m t p f -> p m t f"))
            st["w1t"], st["w2t"], st["tapt"] = w1t, w2t, tapt
            st["b1_col"] = col_tile(b1[l, si], KX, tag="ffcol")
            st["b2_col"] = col_tile(b2[l, si], KE, tag="ffcol")
            st["B_col"] = col_tile(convB[l, si], KX, tag="ffcol")
            st["g_col"] = col_tile(lng[l, 3 if s == "b" else 4], KE,
                                   tag="lncol")
            st["bb_col"] = col_tile(lnb[l, 3 if s == "b" else 4], KE,
                                    tag="lncol")
            st["hts"] = [None] * NC
            st["xts"] = [None] * NC
            return st

        def ffn_h_pair(sts, srcs, ci):
            for st, src in zip(sts, srcs):
                xt = load_x_chunk(src, ci)
                st["xts"][ci] = xt
                ht = pb.tile([128, KX, C + 2], F16, tag="ht")
                if ci == 0:
                    v.memset(ht[:, :, 0:1], 0.0)
                st["hts"][ci] = ht
            for m in range(KX):
                for st in sts:
                    ht, xt = st["hts"][ci], st["xts"][ci]
                    ph = ps.tile([128, C], F32, tag="mm")
                    for k in range(KE):
                        mm(ph, st["w1t"][:, k, m * 128:(m + 1) * 128],
                           xt[:, k, :], start=(k == 0), stop=(k == KE - 1))
                    sc.activation(ht[:, m, 1:C + 1], ph, AF.Relu,
                                  bias=st["b1_col"][:, m:m + 1])
                    v.tensor_scalar_min(ht[:, m, 1:C + 1], ht[:, m, 1:C + 1],
                                        6.0)
            for st in sts:
                ht = st["hts"][ci]
                prev = st["hts"][ci - 1] if ci > 0 else None
                if prev is not None:
                    v.tensor_copy(prev[:, :, C + 1:C + 2], ht[:, :, 1:2])
                    v.tensor_copy(ht[:, :, 0:1], prev[:, :, C:C + 1])
                if ci == NC - 1:
                    v.memset(ht[:, :, C + 1:C + 2], 0.0)

        def ffn_tail_pair(sts, dsts, ci):
            h2s = []
            for st in sts:
                h2 = pb.tile([128, KX, C], F16, tag="h2", bufs=2, name="h2")
                h2s.append(h2)
            for m in range(KX):
                for st, h2 in zip(sts, h2s):
                    ht = st["hts"][ci]
                    pacc = ps.tile([128, C], F32, tag="mm")
                    for t in range(3):
                        mm(pacc, st["tapt"][:, m, t, :], ht[:, m, t:t + C],
                           start=(t == 0), stop=(t == 2))
                    rel = pc.tile([128, C], F16, tag="c1")
                    sc.activation(rel, pacc, AF.Relu,
                                  bias=st["B_col"][:, m:m + 1])
                    v.tensor_scalar_min(h2[:, m, :], rel, 6.0)
            rts = []
            for st, h2 in zip(sts, h2s):
                rt = pa.tile([128, KE, C], F16, tag="a4", name="rt")
                rts.append(rt)
            for m in range(KE):
                for st, h2, rt in zip(sts, h2s, rts):
                    pw = ps.tile([128, C], F32, tag="mm")
                    for k in range(KX):
                        mm(pw, st["w2t"][:, k, m * 128:(m + 1) * 128],
                           h2[:, k, :], start=(k == 0), stop=(k == KX - 1))
                    sc.activation(rt[:, m, :], pw, AF.Identity,
                                  bias=st["b2_col"][:, m:m + 1])
            jobs = []
            for st, rt, dst in zip(sts, rts, dsts):
                v.tensor_tensor(out=rt, in0=rt, in1=st["xts"][ci], op=ALU.add)
                outt = pa.tile([128, KE, C], F16, tag="a4", name="outt")
                jobs.append((rt, st["g_col"], st["bb_col"], outt))
            ln_pair(jobs)
            for j, dst in zip(jobs, dsts):
                store_chunk(dst, ci, j[3])
            for st in sts:
                st["hts"][ci] = st["xts"][ci] = None

        # ---- layers ----
        for l in range(L):
            bsrc = rs["b", 0] if l == 0 else rs["b", (l - 1, 3)]
            lsrc = rs["l", 0] if l == 0 else rs["l", (l - 1, 3)]

            PHASES.append((f"attnA{l}.alpha", len(nc.inst_map)))
            stA = [attn_setup(l, 0), attn_setup(l, 1)]
            for ci in range(NC):
                alpha_pair_step(stA, [bsrc, lsrc], ci)
            for st in stA:
                alpha_fin(st)
            PHASES.append((f"attnA{l}.beta", len(nc.inst_map)))
            tailA = make_self_tail_pair(l, [rs["b", (l, 1)], rs["l", (l, 1)]])
            for ci in range(NC):
                beta_pair_step(stA, [bsrc, lsrc], ci, [tailA])

            PHASES.append((f"attnB{l}.alpha", len(nc.inst_map)))
            b1d, l1d = rs["b", (l, 1)], rs["l", (l, 1)]
            stB = [attn_setup(l, 2), attn_setup(l, 3)]
            for ci in range(NC):
                alpha_pair_step(stB, [l1d, b1d], ci)
            for st in stB:
                alpha_fin(st)
            PHASES.append((f"attnB{l}.beta", len(nc.inst_map)))
            tailB = make_cross_tail_pair(l, [rs["b", (l, 2)],
                                             rs["l", (l, 2)]])
            for ci in range(NC):
                beta_pair_step(stB, [b1d, l1d], ci, [tailB])

            PHASES.append((f"ffn{l}", len(nc.inst_map)))
            stF = [ffn_setup(l, "b"), ffn_setup(l, "l")]
            fsrc = [rs["b", (l, 2)], rs["l", (l, 2)]]
            fdst = [rs["b", (l, 3)], rs["l", (l, 3)]]
            ffn_h_pair(stF, fsrc, 0)
            for ci in range(1, NC):
                ffn_h_pair(stF, fsrc, ci)
                ffn_tail_pair(stF, fdst, ci - 1)
            ffn_tail_pair(stF, fdst, NC - 1)

        PHASES.append(("final", len(nc.inst_map)))
        # ---- final head ----
        fw1t = wq.tile([128, 2 * KE, E2], F16, tag="w1")
        nc.sync.dma_start(out=fw1t,
                          in_=fw1.rearrange("(k p) g -> p k g", p=128))
        fw2t = wq.tile([128, 2, E], F16, tag="gw")
        nc.sync.dma_start(out=fw2t,
                          in_=fw2.rearrange("(k p) e -> p k e", p=128))
        rw1t = wq.tile([128, KE, E4], F16, tag="gw")
        nc.sync.dma_start(out=rw1t,
                          in_=rw1.rearrange("(k p) g -> p k g", p=128))
        rw2t = wrow.tile([128, E8], F16, tag="row2")
        nc.sync.dma_start(out=rw2t, in_=rw2)
        rw3t = wrow.tile([E8, 16], F16, tag="row2")
        nc.sync.dma_start(out=rw3t, in_=rw3p)
        rb3_row = wrow.tile([1, 16], F16, tag="row")
        nc.sync.dma_start(out=rb3_row, in_=rb3p)
        fb1_col = col_tile(fb1, 2, tag="fcol")
        fb2_col = col_tile(fb2, KE, tag="fcol")
        flng_col = col_tile(flng, KE, tag="fcol")
        flnb_col = col_tile(flnb, KE, tag="fcol")
        rb1_col = col_tile(rb1, 1, tag="fcol")
        rb2_col = wcol.tile([E8, 1], F32, tag="fcol")
        nc.sync.dma_start(out=rb2_col, in_=rb2[:, None])
        out_ap = out_dram.ap()

        bsrc, lsrc = rs["b", (L - 1, 3)], rs["l", (L - 1, 3)]
        for ci in range(NC):
            xb = load_x_chunk(bsrc, ci)
            xl = load_x_chunk(lsrc, ci)
            f1t = [pc.tile([128, C], F16, tag="c1", name=f"f1t{_i}")
                   for _i in range(2)]
            for m in range(2):
                pf = ps.tile([128, C], F32, tag="mm")
                for k in range(2 * KE):
                    rhs = xb[:, k, :] if k < KE else xl[:, k - KE, :]
                    mm(pf, fw1t[:, k, m * 128:(m + 1) * 128], rhs,
                       start=(k == 0), stop=(k == 2 * KE - 1))
                f1f = pc.tile([128, C], F16, tag="c1")
                v.tensor_scalar(out=f1f, in0=pf, scalar1=fb1_col[:, m:m + 1],
                                scalar2=0.0, op0=ALU.add, op1=ALU.max)
                v.tensor_scalar_min(f1t[m], f1f, 6.0)
            ft = pa.tile([128, KE, C], F16, tag="a4")
            for m in range(KE):
                pf2 = ps.tile([128, C], F32, tag="mm")
                for k in range(2):
                    mm(pf2, fw2t[:, k, m * 128:(m + 1) * 128],
                       f1t[k], start=(k == 0), stop=(k == 1))
                sc.activation(ft[:, m, :], pf2, AF.Identity,
                              bias=fb2_col[:, m:m + 1])
            frt = pa.tile([128, KE, C], F16, tag="a4")
            ln_pair([(ft, flng_col, flnb_col, frt)], relu=True, apply_dve=True)
            p1 = ps.tile([128, C], F32, tag="mm")
            for k in range(KE):
                mm(p1, rw1t[:, k, :], frt[:, k, :], start=(k == 0),
                   stop=(k == KE - 1))
            h1f = pc.tile([128, C], F16, tag="c1")
            v.tensor_scalar(out=h1f, in0=p1, scalar1=rb1_col[:, 0:1],
                            scalar2=0.0, op0=ALU.add, op1=ALU.max)
            h1t = pc.tile([128, C], F16, tag="c1")
            v.tensor_scalar_min(h1t, h1f, 6.0)
            p2 = ps.tile([E8, C], F32, tag="mm")
            mm(p2, rw2t, h1t, start=True, stop=True)
            h2f = pc.tile([E8, C], F16, tag="c1")
            sc.activation(h2f, p2, AF.Relu, bias=rb2_col[:, 0:1])
            h2t = pc.tile([E8, C], F16, tag="c1")
            v.tensor_scalar_min(h2t, h2f, 6.0)
            ot = pc.tile([128, NTT, c.OUT], F32, tag="c2", bufs=2)
            for tt in range(NTT):
                p3 = ps.tile([128, 16], F32, tag="mm")
                mm(p3, h2t[:, tt * 128:(tt + 1) * 128], rw3t,
                   start=True, stop=False)
                mm(p3, ONES_ROW[:, 0:128], rb3_row, start=False, stop=True)
                sc.activation(ot[:, tt, :], p3[:, 0:c.OUT], AF.Copy)
            nc.sync.dma_start(
                out=out_ap[ci * C:(ci + 1) * C, :].rearrange(
                    "(tt p) o -> p tt o", p=128),
                in_=ot)

    return din, out_dram


# ======================================================================
# kernel() entry point: full inputs in, full outputs out (8-core SPMD).
# ======================================================================
import concourse.bacc as _bacc
from concourse.bass_utils import run_bass_kernel_spmd as _run_spmd

_N_CORES = 8
_CACHE = {}


def _get_nc():
    if "nc" not in _CACHE:
        nc = _bacc.Bacc("TRN2", target_bir_lowering=False, debug=False)
        build(nc, Cfg())
        nc.finalize()
        _CACHE["nc"] = nc
    return _CACHE["nc"]


def kernel(**inputs):
    nc = _get_nc()
    cfg = Cfg()
    arr = {k: np.asarray(v) for k, v in inputs.items()}
    consts = host_constants(cfg, arr)
    shared = {k: a for k, a in consts.items()
              if k not in ("body_feats", "limb_feats")}
    in_maps = []
    for i in range(_N_CORES):
        m = dict(shared)
        m["body_feats"] = np.ascontiguousarray(consts["body_feats"][i])
        m["limb_feats"] = np.ascontiguousarray(consts["limb_feats"][i])
        in_maps.append(m)
    res = run_kernel_spmd_cached(nc, in_maps)
    out = np.stack([res[i]["out"] for i in range(_N_CORES)], axis=0)
    return out.astype(np.float32)


def run_kernel_spmd_cached(nc, in_maps, **kw):
    r = _run_spmd(nc, in_maps, list(range(_N_CORES)), **kw)
    _CACHE["last_result"] = r
    return r.results


# revision 38
# speedup vs baseline: 1.0080x; 1.0080x over previous
"""Dual-stream linear-attention transformer kernel (per-core), v3.

v3: per-m interleaved emission across the body/limb pair (ACT has no
exec-queue lookahead, so fine-grained alternation is what keeps it fed),
LN stats pairs share one PSUM tile (rows 0/1), fp16 operands throughout.
See v2 notes below.

  - fp16 matmul operands + activations + residual DRAM (fp32 PSUM/stats).
  - q/k/v low-rank projections premultiplied on host to single [E,E]
    mats; k/v produced directly token-major by using x as lhsT.
  - depthwise conv on PE via host-built diagonal tap matrices (BN scale
    folded in) against halo-padded h tiles.
  - attention denominator scaled by 1/64 so fp16 reciprocals stay in
    the normal range (1/64 folded into the bd kv blocks).

Layouts:
  - layout 1: [E, N] feature-major; SBUF tiles [128, KE, C].
  - layout 2 (k/v only): [tok, E] token-major.
  - Residuals in internal DRAM as fp16 [E, N] -> p k n.
"""

from dataclasses import dataclass
from contextlib import ExitStack

import numpy as np

import concourse.bass as bass
import concourse.mybir as mybir
import concourse.tile as tile

F32 = mybir.dt.float32
F16 = mybir.dt.float16
AF = mybir.ActivationFunctionType
ALU = mybir.AluOpType

LN_EPS = 1e-5
BN_EPS = 1e-5
DEN_SCALE = 1.0 / 64.0


@dataclass
class Cfg:
    N: int = 2048
    E: int = 512
    R: int = 256
    X: int = 1024
    H: int = 8
    L: int = 3
    OUT: int = 15
    C: int = 512

    @property
    def KE(self):
        return self.E // 128

    @property
    def KX(self):
        return self.X // 128

    @property
    def NC(self):
        return self.N // self.C

    @property
    def NTT(self):
        return self.C // 128


def host_constants(cfg, inputs):
    """Precompute fp16 weights / fused constants on host."""
    c = cfg
    f = lambda a: np.ascontiguousarray(a, dtype=np.float32)
    h = lambda a: np.ascontiguousarray(a, dtype=np.float16)
    dw, uw = f(inputs["dw"]), f(inputs["uw"])
    qkvw = np.einsum("latir,latrj->latij", dw, uw)
    out = {
        "body_feats": h(inputs["body_feats"]),
        "limb_feats": h(inputs["limb_feats"]),
        "qkvw": h(qkvw),
        "ub": f(inputs["ub"]),
        "ub16": h(inputs["ub"]),
        "ow": h(inputs["ow"]),
        "ob": f(inputs["ob"]),
        "w1": h(inputs["w1"]),
        "b1": f(inputs["b1"]),
        "w2": h(inputs["w2"]),
        "b2": f(inputs["b2"]),
        "lng": f(inputs["lng"]),
        "lnb": f(inputs["lnb"]),
        "gw1": h(inputs["gw1"]),
        "gb1": f(inputs["gb1"]),
        "gwd": h(f(inputs["gw2"])[:, :, 0] - f(inputs["gw2"])[:, :, 1]),
        "gb2d": f(f(inputs["gb2"])[:, 0:1] - f(inputs["gb2"])[:, 1:2]),
        "fw1": h(inputs["fw1"]),
        "fb1": f(inputs["fb1"]),
        "fw2": h(inputs["fw2"]),
        "fb2": f(inputs["fb2"]),
        "flng": f(inputs["flng"]),
        "flnb": f(inputs["flnb"]),
        "rw1": h(inputs["rw1"]),
        "rb1": f(inputs["rb1"]),
        "rw2": h(inputs["rw2"]),
        "rb2": f(inputs["rb2"]),
    }
    rw3 = np.zeros((c.E // 8, 16), np.float16)
    rw3[:, : c.OUT] = f(inputs["rw3"])
    out["rw3p"] = rw3
    rb3 = np.zeros((1, 16), np.float16)
    rb3[0, : c.OUT] = f(inputs["rb3"])
    out["rb3p"] = rb3
    rsq = 1.0 / np.sqrt(1.0 + BN_EPS)
    A = f(inputs["bng"]) * rsq
    cw, cb = f(inputs["cw"]), f(inputs["cb"])
    taps = np.zeros((c.L, 2, c.KX, 3, 128, 128), np.float16)
    idx = np.arange(128)
    for t in range(3):
        wA = (cw[:, :, :, t] * A).reshape(c.L, 2, c.KX, 128)
        taps[:, :, :, t, idx, idx] = wA.astype(np.float16)
    out["taps"] = taps
    out["convB"] = f(cb * A + f(inputs["bnb"]))
    out["ident"] = np.eye(128, dtype=np.float16)
    out["ones16"] = np.ones((128, 128), np.float16)
    E, H = c.E, c.H
    dh = E // H
    hmask = np.zeros((E, H), np.float16)
    for ff in range(E):
        hmask[ff, ff // dh] = 1.0
    out["hmask"] = hmask
    out["cmask"] = np.ascontiguousarray(hmask.T)
    return out


PHASES = []


def build(nc, cfg):
    c = cfg
    E, X, H, N, C, L = c.E, c.X, c.H, c.N, c.C, c.L
    KE, KX, NC, NTT = c.KE, c.KX, c.NC, c.NTT
    E4, E2, E8 = E // 4, E // 2, E // 8

    din = {}

    def inp(name, shape, dt):
        din[name] = nc.dram_tensor(name, list(shape), dt, kind="ExternalInput")
        return din[name].ap()

    body_feats = inp("body_feats", (N, E), F16)
    limb_feats = inp("limb_feats", (N, E), F16)
    qkvw = inp("qkvw", (L, 4, 3, E, E), F16)
    ub = inp("ub", (L, 4, 3, E), F32)
    ub16 = inp("ub16", (L, 4, 3, E), F16)
    ow = inp("ow", (L, 4, E, E), F16)
    ob = inp("ob", (L, 4, E), F32)
    w1 = inp("w1", (L, 2, E, X), F16)
    b1 = inp("b1", (L, 2, X), F32)
    w2 = inp("w2", (L, 2, X, E), F16)
    b2 = inp("b2", (L, 2, E), F32)
    taps = inp("taps", (L, 2, KX, 3, 128, 128), F16)
    convB = inp("convB", (L, 2, X), F32)
    lng = inp("lng", (L, 5, E), F32)
    lnb = inp("lnb", (L, 5, E), F32)
    gw1 = inp("gw1", (L, 2 * E, E4), F16)
    gb1 = inp("gb1", (L, E4), F32)
    gwd = inp("gwd", (L, E4), F16)
    gb2d = inp("gb2d", (L, 1), F32)
    fw1 = inp("fw1", (2 * E, E2), F16)
    fb1 = inp("fb1", (E2,), F32)
    fw2 = inp("fw2", (E2, E), F16)
    fb2 = inp("fb2", (E,), F32)
    flng = inp("flng", (E,), F32)
    flnb = inp("flnb", (E,), F32)
    rw1 = inp("rw1", (E, E4), F16)
    rb1 = inp("rb1", (E4,), F32)
    rw2 = inp("rw2", (E4, E8), F16)
    rb2 = inp("rb2", (E8,), F32)
    rw3p = inp("rw3p", (E8, 16), F16)
    rb3p = inp("rb3p", (1, 16), F16)
    ident_in = inp("ident", (128, 128), F16)
    ones_in = inp("ones16", (128, 128), F16)
    hmask_in = inp("hmask", (E, H), F16)
    cmask_in = inp("cmask", (H, E), F16)

    out_dram = nc.dram_tensor("out", [N, c.OUT], F32, kind="ExternalOutput")

    def idram(name):
        return [nc.dram_tensor(f"{name}_c{ci}", [E, C], F16).ap().rearrange(
            "(k p) n -> p k n", p=128) for ci in range(NC)]

    rs = {}
    for s in ("b", "l"):
        rs[s, 0] = idram(f"r{s}0")
        for l in range(L):
            for st in (1, 2, 3):
                rs[s, (l, st)] = idram(f"r{s}_{l}_{st}")

    lowp = nc.allow_low_precision("fp16 activations within rel-err budget")

    with tile.TileContext(nc) as tc, ExitStack() as ctx, lowp:
        p_ = ctx.enter_context
        cst = p_(tc.tile_pool(name="cst", bufs=1))
        wq = p_(tc.tile_pool(name="wq", bufs=2))       # big weights
        wcol = p_(tc.tile_pool(name="wcol", bufs=8))   # bias cols
        wrow = p_(tc.tile_pool(name="wrow", bufs=2))   # bias rows
        pa = p_(tc.tile_pool(name="pa", bufs=10))      # 4KB fp16 act tiles
        pb = p_(tc.tile_pool(name="pb", bufs=4))       # ffn h tiles
        pc = p_(tc.tile_pool(name="pc", bufs=9))       # small tiles
        pat = p_(tc.tile_pool(name="pat", bufs=2))     # attn persistents
        ps = p_(tc.tile_pool(name="ps", bufs=5, space="PSUM"))
        pskv = p_(tc.tile_pool(name="pskv", bufs=1, space="PSUM"))
        psst = p_(tc.tile_pool(name="psst", bufs=2, space="PSUM"))

        v, sc, gp = nc.vector, nc.scalar, nc.gpsimd

        def mm(out, lhsT, rhs, start, stop):
            nc.tensor.matmul(out, lhsT, rhs, start=start, stop=stop)

        # ---- constants ----
        ident_t = cst.tile([128, 128], F16, tag="ident")
        nc.sync.dma_start(out=ident_t, in_=ident_in)
        ones_t = cst.tile([128, 128], F16, tag="ones")
        nc.sync.dma_start(out=ones_t, in_=ones_in)
        hmask_t = cst.tile([128, KE, H], F16, tag="hmask")
        nc.sync.dma_start(out=hmask_t,
                          in_=hmask_in.rearrange("(k p) h -> p k h", p=128))
        cmask_t = cst.tile([H, KE, 128], F16, tag="cmask")
        nc.sync.dma_start(out=cmask_t,
                          in_=cmask_in.rearrange("h (k p) -> h k p", p=128))
        ONES_COL = ones_t[:, 0:1]
        ONES_ROW = ones_t[0:1, :]
        eps_den = cst.tile([8, 1], F32, tag="epsd")
        v.memset(eps_den, 1e-6 * DEN_SCALE)
        eps_ln = cst.tile([1, 1], F32, tag="epsl")
        v.memset(eps_ln, LN_EPS)

        def col_tile(src_ap, m, tag="col"):
            t = wcol.tile([128, m], F32, tag=tag)
            nc.sync.dma_start(out=t, in_=src_ap.rearrange("(m p) -> p m", p=128))
            return t

        def ln_pair(jobs, relu=False, apply_dve=False):
            """LayerNorm over features, pair-interleaved.
            jobs: list of (xs, g_col, b_col, outt); xs/outt fp16
            [128, KE, C]; outt doubles as x^2 scratch."""
            for xs, _, _, outt in jobs:
                v.tensor_tensor(out=outt, in0=xs, in1=xs, op=ALU.mult)
            pst = []
            for xs, _, _, outt in jobs:
                pp_s = psst.tile([8, C], F32, tag="st", name="lnst")
                pp_q = psst.tile([8, C], F32, tag="st", name="lnsq")
                for m in range(KE):
                    mm(pp_s[0:1, :], ONES_COL, xs[:, m, :], start=(m == 0),
                       stop=(m == KE - 1))
                for m in range(KE):
                    mm(pp_q[0:1, :], ONES_COL, outt[:, m, :], start=(m == 0),
                       stop=(m == KE - 1))
                pst.append((pp_s, pp_q))
            stts = []
            for (xs, _, _, _), (pp_s, pp_q) in zip(jobs, pst):
                arow = pc.tile([1, C], F32, tag="s2", bufs=4, name="arow")
                brow = pc.tile([1, C], F32, tag="s2", bufs=4, name="brow")
                sc.activation(arow, pp_s[0:1, :], AF.Copy, scale=1.0 / E)
                sc.activation(brow, pp_q[0:1, :], AF.Copy, scale=1.0 / E)
                trow2 = pc.tile([1, C], F32, tag="s2", bufs=4)
                sc.activation(trow2, arow, AF.Square)
                v.tensor_tensor(out=brow, in0=brow, in1=trow2,
                                op=ALU.subtract)
                sc.activation(brow, brow, AF.Sqrt, bias=eps_ln[0:1, 0:1])
                stt = pc.tile([1, 2, C], F16, tag="s2", bufs=4)
                v.reciprocal(out=stt[:, 0, :], in_=brow)
                v.tensor_tensor(out=stt[:, 1, :], in0=arow, in1=stt[:, 0, :],
                                op=ALU.mult)
                stts.append(stt)
            sbs = []
            for stt in stts:
                bc_s = ps.tile([128, C], F32, tag="mm")
                mm(bc_s, ones_t[0:1, :], stt[0:1, 0, :], start=True, stop=True)
                bc_t = ps.tile([128, C], F32, tag="mm")
                mm(bc_t, ones_t[0:1, :], stt[0:1, 1, :], start=True, stop=True)
                sb = pc.tile([128, 2, C], F16, tag="c2", bufs=2)
                sc.activation(sb[:, 0, :], bc_s, AF.Copy)
                sc.activation(sb[:, 1, :], bc_t, AF.Copy)
                sbs.append(sb)
            fn = AF.Relu if relu else AF.Identity
            for m in range(KE):
                for (xs, g_col, b_col, outt), sb in zip(jobs, sbs):
                    t1 = pc.tile([128, C], F16, tag="c1")
                    v.tensor_tensor(out=t1, in0=xs[:, m, :], in1=sb[:, 0, :],
                                    op=ALU.mult)
                    v.tensor_tensor(out=t1, in0=t1, in1=sb[:, 1, :],
                                    op=ALU.subtract)
                    if apply_dve:
                        v.tensor_scalar(out=t1, in0=t1,
                                        scalar1=g_col[:, m:m + 1],
                                        scalar2=b_col[:, m:m + 1],
                                        op0=ALU.mult, op1=ALU.add)
                        if relu:
                            v.tensor_scalar_max(outt[:, m, :], t1, 0.0)
                        else:
                            v.tensor_copy(outt[:, m, :], t1)
                    else:
                        sc.activation(outt[:, m, :], t1, fn,
                                      bias=b_col[:, m:m + 1],
                                      scale=g_col[:, m:m + 1])

        def load_x_chunk(dram_l1, ci, tag="a4"):
            xt = pa.tile([128, KE, C], F16, tag=tag)
            nc.sync.dma_start(out=xt, in_=dram_l1[ci])
            return xt

        def store_chunk(dram_l1, ci, t):
            gp.dma_start(out=dram_l1[ci], in_=t)

        # ---- entry transpose (interleaved) ----
        def entry_tile(x_ap, dst, ttk):
            x2 = pa.tile([128, E], F16, tag="a4")
            nc.sync.dma_start(out=x2, in_=x_ap[ttk * 128:(ttk + 1) * 128, :])
            xt = pa.tile([128, KE, 128], F16, tag="a4")
            for f in range(KE):
                pt = ps.tile([128, 128], F16, tag="mm")
                nc.tensor.transpose(pt, x2[:, f * 128:(f + 1) * 128], ident_t)
                if f % 2 == 0:
                    sc.activation(xt[:, f, :], pt, AF.Copy)
                else:
                    v.tensor_copy(xt[:, f, :], pt)
            tl = ttk % NTT
            nc.sync.dma_start(out=dst[ttk // NTT][:, :, tl * 128:(tl + 1) * 128],
                              in_=xt)

        PHASES.append(("entry", len(nc.inst_map)))
        for ttk in range(N // 128):
            entry_tile(body_feats, rs["b", 0], ttk)
            entry_tile(limb_feats, rs["l", 0], ttk)

        # ---- linear attention (pairs) ----
        def attn_setup(l, a):
            st = {}
            kvw = wq.tile([128, KE, 2, E], F16, tag="kvw")
            for t3 in (1, 2):
                nc.sync.dma_start(
                    out=kvw[:, :, t3 - 1, :],
                    in_=qkvw[l, a, t3].rearrange("(k p) e -> p k e", p=128))
            qw = wq.tile([128, KE, E], F16, tag="qw")
            nc.sync.dma_start(
                out=qw, in_=qkvw[l, a, 0].rearrange("(k p) e -> p k e", p=128))
            owt = wq.tile([128, KE, E], F16, tag="ow")
            nc.sync.dma_start(
                out=owt, in_=ow[l, a].rearrange("(k p) e -> p k e", p=128))
            st["kvw"], st["qw"], st["owt"] = kvw, qw, owt
            st["ubq_col"] = col_tile(ub[l, a, 0], KE)
            ubkv = wrow.tile([1, 2, E], F16, tag="row")
            nc.sync.dma_start(out=ubkv[:, 0, :], in_=ub16[l, a, 1][None, :])
            nc.sync.dma_start(out=ubkv[:, 1, :], in_=ub16[l, a, 2][None, :])
            st["ubkv"] = ubkv
            st["ob_col"] = col_tile(ob[l, a], KE)
            st["kv_acc"] = pat.tile([128, 4, 258], F32, tag="kva",
                                    name="kv_acc")
            return st

        def alpha_pair_step(sts, srcs, ci):
            xts = [load_x_chunk(src, ci) for src in srcs]
            work = []
            for st, xt in zip(sts, xts):
                k2f = pa.tile([128, NTT, E], F16, tag="a4", name="k2f")
                v2x = pa.tile([128, NTT, 2, 258], F16, tag="a4", name="v2x")
                v.memset(v2x[:, :, :, 256:258], 1.0)
                work.append((st, xt, k2f, v2x))
            for tt in range(NTT):
                for st, xt, k2f, v2x in work:
                    kvw = st["kvw"]
                    pk = ps.tile([128, E], F32, tag="mm")
                    pv = ps.tile([128, E], F32, tag="mm")
                    for k in range(KE):
                        lx = xt[:, k, tt * 128:(tt + 1) * 128]
                        mm(pk, lx, kvw[:, k, 0, :], start=(k == 0), stop=False)
                        mm(pv, lx, kvw[:, k, 1, :], start=(k == 0), stop=False)
                    mm(pk, ONES_ROW, st["ubkv"][:, 0, :], start=False,
                       stop=True)
                    mm(pv, ONES_ROW, st["ubkv"][:, 1, :], start=False,
                       stop=True)
                    ee = pc.tile([128, E], F16, tag="c1")
                    rr = pc.tile([128, E], F16, tag="c1")
                    sc.activation(ee, pk, AF.Exp)
                    v.tensor_scalar_max(rr, pk, 0.0)
                    v.tensor_scalar_min(ee, ee, 1.0)
                    v.tensor_tensor(out=k2f[:, tt, :], in0=ee, in1=rr,
                                    op=ALU.add)
                    sc.activation(v2x[:, tt, 0, 0:256], pv[:, 0:256], AF.Copy)
                    sc.activation(v2x[:, tt, 1, 0:256], pv[:, 256:512],
                                  AF.Copy)
            for p in range(4):
                for st, xt, k2f, v2x in work:
                    pkv = pskv.tile([128, 258], F32, tag="kv")
                    for tt in range(NTT):
                        mm(pkv, k2f[:, tt, p * 128:(p + 1) * 128],
                           v2x[:, tt, p // 2, :],
                           start=(tt == 0), stop=(tt == NTT - 1))
                    kv_acc = st["kv_acc"]
                    if ci == 0:
                        sc.activation(kv_acc[:, p, :], pkv, AF.Copy)
                    else:
                        v.tensor_tensor(out=kv_acc[:, p, :],
                                        in0=kv_acc[:, p, :], in1=pkv,
                                        op=ALU.add)

        def alpha_fin(st):
            kv_acc = st["kv_acc"]
            bd = pat.tile([128, KE, 128], F16, tag="bd")
            v.memset(bd, 0.0)
            for p in range(4):
                h0c = (2 * p % 4) * 64
                h1c = ((2 * p + 1) % 4) * 64
                v.tensor_scalar_mul(bd[0:64, p, 0:64],
                                    kv_acc[0:64, p, h0c:h0c + 64], DEN_SCALE)
                v.tensor_scalar_mul(bd[64:128, p, 64:128],
                                    kv_acc[64:128, p, h1c:h1c + 64], DEN_SCALE)
            kmm = pat.tile([128, KE, H], F16, tag="km")
            for k in range(KE):
                v.tensor_scalar(out=kmm[:, k, :], in0=hmask_t[:, k, :],
                                scalar1=kv_acc[:, k, 256:257],
                                scalar2=DEN_SCALE, op0=ALU.mult, op1=ALU.mult)
            st["bd"], st["kmm"] = bd, kmm

        def beta_pair_step(sts, srcs, ci, tails):
            xqs = [load_x_chunk(src, ci) for src in srcs]
            qfs = [pa.tile([128, KE, C], F16, tag="a4", name="qf")
                   for _ in sts]
            pds = [psst.tile([8, C], F32, tag="st", name="pd")
                   for _ in sts]
            for m in range(KE):
                for st, xq, qf, pd in zip(sts, xqs, qfs, pds):
                    qw = st["qw"]
                    pq = ps.tile([128, C], F32, tag="mm")
                    for k in range(KE):
                        mm(pq, qw[:, k, m * 128:(m + 1) * 128],
                           xq[:, k, :], start=(k == 0), stop=(k == KE - 1))
                    ee = pc.tile([128, C], F16, tag="c1")
                    rr = pc.tile([128, C], F16, tag="c1")
                    sc.activation(ee, pq, AF.Exp,
                                  bias=st["ubq_col"][:, m:m + 1])
                    v.tensor_scalar(out=rr, in0=pq,
                                    scalar1=st["ubq_col"][:, m:m + 1],
                                    scalar2=0.0, op0=ALU.add, op1=ALU.max)
                    v.tensor_scalar_min(ee, ee, 1.0)
                    v.tensor_tensor(out=qf[:, m, :], in0=ee, in1=rr,
                                    op=ALU.add)
                    mm(pd, st["kmm"][:, m, :], qf[:, m, :], start=(m == 0),
                       stop=(m == KE - 1))
            recs = []
            for st, pd in zip(sts, pds):
                rec = pc.tile([8, C], F16, tag="s2", bufs=4)
                v.reciprocal(out=rec, in_=pd)
                recs.append(rec)
            atts = [pa.tile([128, KE, C], F16, tag="a4", name="att")
                    for _ in sts]
            for m in range(KE):
                for st, qf, att, rec in zip(sts, qfs, atts, recs):
                    pn = ps.tile([128, C], F32, tag="mm")
                    mm(pn, st["bd"][:, m, :], qf[:, m, :], start=True,
                       stop=True)
                    pr = ps.tile([128, C], F32, tag="mm")
                    mm(pr, cmask_t[:, m, :], rec, start=True, stop=True)
                    rb = pc.tile([128, C], F16, tag="c1")
                    sc.activation(rb, pr, AF.Copy)
                    v.tensor_tensor(out=att[:, m, :], in0=pn, in1=rb,
                                    op=ALU.mult)
            projs = [pa.tile([128, KE, C], F16, tag="a4", name="proj")
                     for _ in sts]
            for m in range(KE):
                for st, att, proj in zip(sts, atts, projs):
                    po = ps.tile([128, C], F32, tag="mm")
                    for k in range(KE):
                        mm(po, st["owt"][:, k, m * 128:(m + 1) * 128],
                           att[:, k, :], start=(k == 0), stop=(k == KE - 1))
                    sc.activation(proj[:, m, :], po, AF.Identity,
                                  bias=st["ob_col"][:, m:m + 1])
            tails[0](ci, projs, xqs)

        # ---- tails (pair) ----
        def make_self_tail_pair(l, dsts):
            cols = []
            for i, s in enumerate(("b", "l")):
                g_col = col_tile(lng[l, i], KE, tag="lncol")
                b_col = col_tile(lnb[l, i], KE, tag="lncol")
                cols.append((g_col, b_col))

            def tail(ci, projs, xqs):
                jobs = []
                for (g_col, b_col), proj, xq, dst in zip(cols, projs, xqs,
                                                         dsts):
                    v.tensor_tensor(out=proj, in0=proj, in1=xq, op=ALU.add)
                for (g_col, b_col), proj, xq, dst in zip(cols, projs, xqs,
                                                         dsts):
                    outt = pa.tile([128, KE, C], F16, tag="a4", name="outt")
                    jobs.append((proj, g_col, b_col, outt))
                ln_pair(jobs)
                for (j, dst) in zip(jobs, dsts):
                    store_chunk(dst, ci, j[3])

            return tail

        def make_cross_tail_pair(l, dsts):
            gw1t = wq.tile([128, 2 * KE, E4], F16, tag="gw")
            nc.sync.dma_start(out=gw1t,
                              in_=gw1[l].rearrange("(k p) g -> p k g", p=128))
            gwd_col = wcol.tile([128, 1], F16, tag="gwd")
            nc.sync.dma_start(out=gwd_col,
                              in_=gwd[l].rearrange("(m p) -> p m", p=128))
            gb1_col = col_tile(gb1[l], 1, tag="lncol")
            gb2d_t = wcol.tile([1, 1], F32, tag="gb2d")
            nc.sync.dma_start(out=gb2d_t, in_=gb2d[l][None, :])
            g_col = col_tile(lng[l, 2], KE, tag="lncol")
            b_col = col_tile(lnb[l, 2], KE, tag="lncol")

            def tail(ci, projs, xqs):
                bgts = []
                for proj, xq in zip(projs, xqs):
                    pg = ps.tile([128, C], F32, tag="mm")
                    for k in range(2 * KE):
                        rhs = xq[:, k, :] if k < KE else proj[:, k - KE, :]
                        mm(pg, gw1t[:, k, :], rhs, start=(k == 0),
                           stop=(k == 2 * KE - 1))
                    g1f = pc.tile([128, C], F16, tag="c1")
                    sc.activation(g1f, pg, AF.Relu, bias=gb1_col[:, 0:1])
                    g1t = pc.tile([128, C], F16, tag="c1")
                    v.tensor_scalar_min(g1t, g1f, 6.0)
                    pg2 = psst.tile([8, C], F32, tag="st", name="pg2")
                    mm(pg2[0:1, :], gwd_col, g1t, start=True, stop=True)
                    bgf = pc.tile([1, C], F16, tag="s2", bufs=4)
                    sc.activation(bgf, pg2[0:1, :], AF.Sigmoid,
                                  bias=gb2d_t[0:1, 0:1])
                    pbg = ps.tile([128, C], F32, tag="mm")
                    mm(pbg, ones_t[0:1, :], bgf, start=True, stop=True)
                    bgt = pc.tile([128, C], F16, tag="c1")
                    sc.activation(bgt, pbg, AF.Copy)
                    bgts.append(bgt)
                jobs = []
                for proj, xq, bgt in zip(projs, xqs, bgts):
                    mt = pa.tile([128, KE, C], F16, tag="a4", name="mt")
                    v.tensor_tensor(out=mt, in0=xq, in1=proj, op=ALU.subtract)
                    for m in range(KE):
                        v.tensor_tensor(out=mt[:, m, :], in0=mt[:, m, :],
                                        in1=bgt, op=ALU.mult)
                    v.tensor_tensor(out=mt, in0=mt, in1=proj, op=ALU.add)
                    outt = pa.tile([128, KE, C], F16, tag="a4", name="outt")
                    jobs.append((mt, g_col, b_col, outt))
                ln_pair(jobs)
                for (j, dst) in zip(jobs, dsts):
                    store_chunk(dst, ci, j[3])

            return tail

        # ---- FFN pair ----
        def ffn_setup(l, s):
            si = 0 if s == "b" else 1
            st = {}
            w1t = wq.tile([128, KE, X], F16, tag="w1")
            nc.sync.dma_start(
                out=w1t, in_=w1[l, si].rearrange("(k p) x -> p k x", p=128))
            w2t = wq.tile([128, KX, E], F16, tag="w2")
            nc.sync.dma_start(
                out=w2t, in_=w2[l, si].rearrange("(k p) e -> p k e", p=128))
            tapt = wq.tile([128, KX, 3, 128], F16, tag="tp" + s, bufs=1)
            nc.sync.dma_start(out=tapt,
                              in_=taps[l, si].rearrange("m t p f -> p m t f"))
            st["w1t"], st["w2t"], st["tapt"] = w1t, w2t, tapt
            st["b1_col"] = col_tile(b1[l, si], KX, tag="ffcol")
            st["b2_col"] = col_tile(b2[l, si], KE, tag="ffcol")
            st["B_col"] = col_tile(convB[l, si], KX, tag="ffcol")
            st["g_col"] = col_tile(lng[l, 3 if s == "b" else 4], KE,
                                   tag="lncol")
            st["bb_col"] = col_tile(lnb[l, 3 if s == "b" else 4], KE,
                                    tag="lncol")
            st["hts"] = [None] * NC
            st["xts"] = [None] * NC
            return st

        def ffn_h_pair(sts, srcs, ci):
            for st, src in zip(sts, srcs):
                xt = load_x_chunk(src, ci)
                st["xts"][ci] = xt
                ht = pb.tile([128, KX, C + 2], F16, tag="ht")
                if ci == 0:
                    v.memset(ht[:, :, 0:1], 0.0)
                st["hts"][ci] = ht
            for m in range(KX):
                for st in sts:
                    ht, xt = st["hts"][ci], st["xts"][ci]
                    ph = ps.tile([128, C], F32, tag="mm")
                    for k in range(KE):
                        mm(ph, st["w1t"][:, k, m * 128:(m + 1) * 128],
                           xt[:, k, :], start=(k == 0), stop=(k == KE - 1))
                    sc.activation(ht[:, m, 1:C + 1], ph, AF.Relu,
                                  bias=st["b1_col"][:, m:m + 1])
                    v.tensor_scalar_min(ht[:, m, 1:C + 1], ht[:, m, 1:C + 1],
                                        6.0)
            for st in sts:
                ht = st["hts"][ci]
                prev = st["hts"][ci - 1] if ci > 0 else None
                if prev is not None:
                    v.tensor_copy(prev[:, :, C + 1:C + 2], ht[:, :, 1:2])
                    v.tensor_copy(ht[:, :, 0:1], prev[:, :, C:C + 1])
                if ci == NC - 1:
                    v.memset(ht[:, :, C + 1:C + 2], 0.0)

        def ffn_tail_pair(sts, dsts, ci):
            h2s = []
            for st in sts:
                h2 = pb.tile([128, KX, C], F16, tag="h2", bufs=2, name="h2")
                h2s.append(h2)
            for m in range(KX):
                for st, h2 in zip(sts, h2s):
                    ht = st["hts"][ci]
                    pacc = ps.tile([128, C], F32, tag="mm")
                    for t in range(3):
                        mm(pacc, st["tapt"][:, m, t, :], ht[:, m, t:t + C],
                           start=(t == 0), stop=(t == 2))
                    rel = pc.tile([128, C], F16, tag="c1")
                    sc.activation(rel, pacc, AF.Relu,
                                  bias=st["B_col"][:, m:m + 1])
                    v.tensor_scalar_min(h2[:, m, :], rel, 6.0)
            rts = []
            for st, h2 in zip(sts, h2s):
                rt = pa.tile([128, KE, C], F16, tag="a4", name="rt")
                rts.append(rt)
            for m in range(KE):
                for st, h2, rt in zip(sts, h2s, rts):
                    pw = ps.tile([128, C], F32, tag="mm")
                    for k in range(KX):
                        mm(pw, st["w2t"][:, k, m * 128:(m + 1) * 128],
                           h2[:, k, :], start=(k == 0), stop=(k == KX - 1))
                    sc.activation(rt[:, m, :], pw, AF.Identity,
                                  bias=st["b2_col"][:, m:m + 1])
            jobs = []
            for st, rt, dst in zip(sts, rts, dsts):
                v.tensor_tensor(out=rt, in0=rt, in1=st["xts"][ci], op=ALU.add)
                outt = pa.tile([128, KE, C], F16, tag="a4", name="outt")
                jobs.append((rt, st["g_col"], st["bb_col"], outt))
            ln_pair(jobs)
            for j, dst in zip(jobs, dsts):
                store_chunk(dst, ci, j[3])
            for st in sts:
                st["hts"][ci] = st["xts"][ci] = None

        # ---- layers ----
        for l in range(L):
            bsrc = rs["b", 0] if l == 0 else rs["b", (l - 1, 3)]
            lsrc = rs["l", 0] if l == 0 else rs["l", (l - 1, 3)]

            PHASES.append((f"attnA{l}.alpha", len(nc.inst_map)))
            stA = [attn_setup(l, 0), attn_setup(l, 1)]
            for ci in range(NC):
                alpha_pair_step(stA, [bsrc, lsrc], ci)
            for st in stA:
                alpha_fin(st)
            PHASES.append((f"attnA{l}.beta", len(nc.inst_map)))
            tailA = make_self_tail_pair(l, [rs["b", (l, 1)], rs["l", (l, 1)]])
            for ci in range(NC):
                beta_pair_step(stA, [bsrc, lsrc], ci, [tailA])

            PHASES.append((f"attnB{l}.alpha", len(nc.inst_map)))
            b1d, l1d = rs["b", (l, 1)], rs["l", (l, 1)]
            stB = [attn_setup(l, 2), attn_setup(l, 3)]
            for ci in range(NC):
                alpha_pair_step(stB, [l1d, b1d], ci)
            for st in stB:
                alpha_fin(st)
            PHASES.append((f"attnB{l}.beta", len(nc.inst_map)))
            tailB = make_cross_tail_pair(l, [rs["b", (l, 2)],
                                             rs["l", (l, 2)]])
            for ci in range(NC):
                beta_pair_step(stB, [b1d, l1d], ci, [tailB])

            PHASES.append((f"ffn{l}", len(nc.inst_map)))
            stF = [ffn_setup(l, "b"), ffn_setup(l, "l")]
            fsrc = [rs["b", (l, 2)], rs["l", (l, 2)]]
            fdst = [rs["b", (l, 3)], rs["l", (l, 3)]]
            ffn_h_pair(stF, fsrc, 0)
            for ci in range(1, NC):
                ffn_h_pair(stF, fsrc, ci)
                ffn_tail_pair(stF, fdst, ci - 1)
            ffn_tail_pair(stF, fdst, NC - 1)

        PHASES.append(("final", len(nc.inst_map)))
        # ---- final head ----
        fw1t = wq.tile([128, 2 * KE, E2], F16, tag="w1")
        nc.sync.dma_start(out=fw1t,
                          in_=fw1.rearrange("(k p) g -> p k g", p=128))
        fw2t = wq.tile([128, 2, E], F16, tag="gw")
        nc.sync.dma_start(out=fw2t,
                          in_=fw2.rearrange("(k p) e -> p k e", p=128))
        rw1t = wq.tile([128, KE, E4], F16, tag="gw")
        nc.sync.dma_start(out=rw1t,
                          in_=rw1.rearrange("(k p) g -> p k g", p=128))
        rw2t = wrow.tile([128, E8], F16, tag="row2")
        nc.sync.dma_start(out=rw2t, in_=rw2)
        rw3t = wrow.tile([E8, 16], F16, tag="row2")
        nc.sync.dma_start(out=rw3t, in_=rw3p)
        rb3_row = wrow.tile([1, 16], F16, tag="row")
        nc.sync.dma_start(out=rb3_row, in_=rb3p)
        fb1_col = col_tile(fb1, 2, tag="fcol")
        fb2_col = col_tile(fb2, KE, tag="fcol")
        flng_col = col_tile(flng, KE, tag="fcol")
        flnb_col = col_tile(flnb, KE, tag="fcol")
        rb1_col = col_tile(rb1, 1, tag="fcol")
        rb2_col = wcol.tile([E8, 1], F32, tag="fcol")
        nc.sync.dma_start(out=rb2_col, in_=rb2[:, None])
        out_ap = out_dram.ap()

        bsrc, lsrc = rs["b", (L - 1, 3)], rs["l", (L - 1, 3)]
        for cp in range(0, NC, 2):
            prs = []
            for ci in (cp, cp + 1):
                xb = load_x_chunk(bsrc, ci)
                xl = load_x_chunk(lsrc, ci)
                f1t = [pc.tile([128, C], F16, tag="c1", name=f"f1t{_i}")
                       for _i in range(2)]
                prs.append({"ci": ci, "xb": xb, "xl": xl, "f1t": f1t})
            for m in range(2):
                for pr_ in prs:
                    pf = ps.tile([128, C], F32, tag="mm")
                    for k in range(2 * KE):
                        rhs = (pr_["xb"][:, k, :] if k < KE
                               else pr_["xl"][:, k - KE, :])
                        mm(pf, fw1t[:, k, m * 128:(m + 1) * 128], rhs,
                           start=(k == 0), stop=(k == 2 * KE - 1))
                    f1f = pc.tile([128, C], F16, tag="c1")
                    v.tensor_scalar(out=f1f, in0=pf,
                                    scalar1=fb1_col[:, m:m + 1],
                                    scalar2=0.0, op0=ALU.add, op1=ALU.max)
                    v.tensor_scalar_min(pr_["f1t"][m], f1f, 6.0)
            for pr_ in prs:
                pr_["ft"] = pa.tile([128, KE, C], F16, tag="a4", name="ft")
            for m in range(KE):
                for pr_ in prs:
                    pf2 = ps.tile([128, C], F32, tag="mm")
                    for k in range(2):
                        mm(pf2, fw2t[:, k, m * 128:(m + 1) * 128],
                           pr_["f1t"][k], start=(k == 0), stop=(k == 1))
                    sc.activation(pr_["ft"][:, m, :], pf2, AF.Identity,
                                  bias=fb2_col[:, m:m + 1])
            jobs = []
            for pr_ in prs:
                frt = pa.tile([128, KE, C], F16, tag="a4", name="frt")
                pr_["frt"] = frt
                jobs.append((pr_["ft"], flng_col, flnb_col, frt))
            ln_pair(jobs, relu=True)
            for pr_ in prs:
                p1 = ps.tile([128, C], F32, tag="mm")
                for k in range(KE):
                    mm(p1, rw1t[:, k, :], pr_["frt"][:, k, :],
                       start=(k == 0), stop=(k == KE - 1))
                pr_["p1"] = p1
            for pr_ in prs:
                h1f = pc.tile([128, C], F16, tag="c1")
                v.tensor_scalar(out=h1f, in0=pr_["p1"],
                                scalar1=rb1_col[:, 0:1],
                                scalar2=0.0, op0=ALU.add, op1=ALU.max)
                h1t = pc.tile([128, C], F16, tag="c1")
                v.tensor_scalar_min(h1t, h1f, 6.0)
                pr_["p2"] = ps.tile([E8, C], F32, tag="mm", name="p2")
                mm(pr_["p2"], rw2t, h1t, start=True, stop=True)
            for pr_ in prs:
                h2f = pc.tile([E8, C], F16, tag="c1")
                sc.activation(h2f, pr_["p2"], AF.Relu, bias=rb2_col[:, 0:1])
                h2t = pc.tile([E8, C], F16, tag="c1")
                v.tensor_scalar_min(h2t, h2f, 6.0)
                pr_["h2t"] = h2t
            for pr_ in prs:
                ot = pc.tile([128, NTT, c.OUT], F32, tag="c2", bufs=2)
                for tt in range(NTT):
                    p3 = ps.tile([128, 16], F32, tag="mm")
                    mm(p3, pr_["h2t"][:, tt * 128:(tt + 1) * 128], rw3t,
                       start=True, stop=False)
                    mm(p3, ONES_ROW[:, 0:128], rb3_row, start=False, stop=True)
                    sc.activation(ot[:, tt, :], p3[:, 0:c.OUT], AF.Copy)
                ci = pr_["ci"]
                nc.sync.dma_start(
                    out=out_ap[ci * C:(ci + 1) * C, :].rearrange(
                        "(tt p) o -> p tt o", p=128),
                    in_=ot)

    return din, out_dram


# ======================================================================
# kernel() entry point: full inputs in, full outputs out (8-core SPMD).
# ======================================================================
import concourse.bacc as _bacc
from concourse.bass_utils import run_bass_kernel_spmd as _run_spmd

_N_CORES = 8
_CACHE = {}


def _get_nc():
    if "nc" not in _CACHE:
        nc = _bacc.Bacc("TRN2", target_bir_lowering=False, debug=False)
        build(nc, Cfg())
        nc.finalize()
        _CACHE["nc"] = nc
    return _CACHE["nc"]


def kernel(**inputs):
    nc = _get_nc()
    cfg = Cfg()
    arr = {k: np.asarray(v) for k, v in inputs.items()}
    consts = host_constants(cfg, arr)
    shared = {k: a for k, a in consts.items()
              if k not in ("body_feats", "limb_feats")}
    in_maps = []
    for i in range(_N_CORES):
        m = dict(shared)
        m["body_feats"] = np.ascontiguousarray(consts["body_feats"][i])
        m["limb_feats"] = np.ascontiguousarray(consts["limb_feats"][i])
        in_maps.append(m)
    res = run_kernel_spmd_cached(nc, in_maps)
    out = np.stack([res[i]["out"] for i in range(_N_CORES)], axis=0)
    return out.astype(np.float32)


def run_kernel_spmd_cached(nc, in_maps, **kw):
    r = _run_spmd(nc, in_maps, list(range(_N_CORES)), **kw)
    _CACHE["last_result"] = r
    return r.results


# revision 39
# speedup vs baseline: 1.0139x; 1.0058x over previous
"""Dual-stream linear-attention transformer kernel (per-core), v3.

v3: per-m interleaved emission across the body/limb pair (ACT has no
exec-queue lookahead, so fine-grained alternation is what keeps it fed),
LN stats pairs share one PSUM tile (rows 0/1), fp16 operands throughout.
See v2 notes below.

  - fp16 matmul operands + activations + residual DRAM (fp32 PSUM/stats).
  - q/k/v low-rank projections premultiplied on host to single [E,E]
    mats; k/v produced directly token-major by using x as lhsT.
  - depthwise conv on PE via host-built diagonal tap matrices (BN scale
    folded in) against halo-padded h tiles.
  - attention denominator scaled by 1/64 so fp16 reciprocals stay in
    the normal range (1/64 folded into the bd kv blocks).

Layouts:
  - layout 1: [E, N] feature-major; SBUF tiles [128, KE, C].
  - layout 2 (k/v only): [tok, E] token-major.
  - Residuals in internal DRAM as fp16 [E, N] -> p k n.
"""

from dataclasses import dataclass
from contextlib import ExitStack

import numpy as np

import concourse.bass as bass
import concourse.mybir as mybir
import concourse.tile as tile

F32 = mybir.dt.float32
F16 = mybir.dt.float16
AF = mybir.ActivationFunctionType
ALU = mybir.AluOpType

LN_EPS = 1e-5
BN_EPS = 1e-5
DEN_SCALE = 1.0 / 64.0


@dataclass
class Cfg:
    N: int = 2048
    E: int = 512
    R: int = 256
    X: int = 1024
    H: int = 8
    L: int = 3
    OUT: int = 15
    C: int = 512

    @property
    def KE(self):
        return self.E // 128

    @property
    def KX(self):
        return self.X // 128

    @property
    def NC(self):
        return self.N // self.C

    @property
    def NTT(self):
        return self.C // 128


def host_constants(cfg, inputs):
    """Precompute fp16 weights / fused constants on host."""
    c = cfg
    f = lambda a: np.ascontiguousarray(a, dtype=np.float32)
    h = lambda a: np.ascontiguousarray(a, dtype=np.float16)
    dw, uw = f(inputs["dw"]), f(inputs["uw"])
    qkvw = np.einsum("latir,latrj->latij", dw, uw)
    out = {
        "body_feats": h(inputs["body_feats"]),
        "limb_feats": h(inputs["limb_feats"]),
        "qkvw": h(qkvw),
        "ub": f(inputs["ub"]),
        "ub16": h(inputs["ub"]),
        "ow": h(inputs["ow"]),
        "ob": f(inputs["ob"]),
        "w1": h(inputs["w1"]),
        "b1": f(inputs["b1"]),
        "w2": h(inputs["w2"]),
        "b2": f(inputs["b2"]),
        "lng": f(inputs["lng"]),
        "lnb": f(inputs["lnb"]),
        "gw1": h(inputs["gw1"]),
        "gb1": f(inputs["gb1"]),
        "gwd": h(f(inputs["gw2"])[:, :, 0] - f(inputs["gw2"])[:, :, 1]),
        "gb2d": f(f(inputs["gb2"])[:, 0:1] - f(inputs["gb2"])[:, 1:2]),
        "fw1": h(inputs["fw1"]),
        "fb1": f(inputs["fb1"]),
        "fw2": h(inputs["fw2"]),
        "fb2": f(inputs["fb2"]),
        "flng": f(inputs["flng"]),
        "flnb": f(inputs["flnb"]),
        "rw1": h(inputs["rw1"]),
        "rb1": f(inputs["rb1"]),
        "rw2": h(inputs["rw2"]),
        "rb2": f(inputs["rb2"]),
    }
    rw3 = np.zeros((c.E // 8, 16), np.float16)
    rw3[:, : c.OUT] = f(inputs["rw3"])
    out["rw3p"] = rw3
    rb3 = np.zeros((1, 16), np.float16)
    rb3[0, : c.OUT] = f(inputs["rb3"])
    out["rb3p"] = rb3
    rsq = 1.0 / np.sqrt(1.0 + BN_EPS)
    A = f(inputs["bng"]) * rsq
    cw, cb = f(inputs["cw"]), f(inputs["cb"])
    taps = np.zeros((c.L, 2, c.KX, 3, 128, 128), np.float16)
    idx = np.arange(128)
    for t in range(3):
        wA = (cw[:, :, :, t] * A).reshape(c.L, 2, c.KX, 128)
        taps[:, :, :, t, idx, idx] = wA.astype(np.float16)
    out["taps"] = taps
    out["convB"] = f(cb * A + f(inputs["bnb"]))
    out["ident"] = np.eye(128, dtype=np.float16)
    out["ones16"] = np.ones((128, 128), np.float16)
    E, H = c.E, c.H
    dh = E // H
    hmask = np.zeros((E, H), np.float16)
    for ff in range(E):
        hmask[ff, ff // dh] = 1.0
    out["hmask"] = hmask
    out["cmask"] = np.ascontiguousarray(hmask.T)
    return out


PHASES = []


def build(nc, cfg):
    c = cfg
    E, X, H, N, C, L = c.E, c.X, c.H, c.N, c.C, c.L
    KE, KX, NC, NTT = c.KE, c.KX, c.NC, c.NTT
    E4, E2, E8 = E // 4, E // 2, E // 8

    din = {}

    def inp(name, shape, dt):
        din[name] = nc.dram_tensor(name, list(shape), dt, kind="ExternalInput")
        return din[name].ap()

    body_feats = inp("body_feats", (N, E), F16)
    limb_feats = inp("limb_feats", (N, E), F16)
    qkvw = inp("qkvw", (L, 4, 3, E, E), F16)
    ub = inp("ub", (L, 4, 3, E), F32)
    ub16 = inp("ub16", (L, 4, 3, E), F16)
    ow = inp("ow", (L, 4, E, E), F16)
    ob = inp("ob", (L, 4, E), F32)
    w1 = inp("w1", (L, 2, E, X), F16)
    b1 = inp("b1", (L, 2, X), F32)
    w2 = inp("w2", (L, 2, X, E), F16)
    b2 = inp("b2", (L, 2, E), F32)
    taps = inp("taps", (L, 2, KX, 3, 128, 128), F16)
    convB = inp("convB", (L, 2, X), F32)
    lng = inp("lng", (L, 5, E), F32)
    lnb = inp("lnb", (L, 5, E), F32)
    gw1 = inp("gw1", (L, 2 * E, E4), F16)
    gb1 = inp("gb1", (L, E4), F32)
    gwd = inp("gwd", (L, E4), F16)
    gb2d = inp("gb2d", (L, 1), F32)
    fw1 = inp("fw1", (2 * E, E2), F16)
    fb1 = inp("fb1", (E2,), F32)
    fw2 = inp("fw2", (E2, E), F16)
    fb2 = inp("fb2", (E,), F32)
    flng = inp("flng", (E,), F32)
    flnb = inp("flnb", (E,), F32)
    rw1 = inp("rw1", (E, E4), F16)
    rb1 = inp("rb1", (E4,), F32)
    rw2 = inp("rw2", (E4, E8), F16)
    rb2 = inp("rb2", (E8,), F32)
    rw3p = inp("rw3p", (E8, 16), F16)
    rb3p = inp("rb3p", (1, 16), F16)
    ident_in = inp("ident", (128, 128), F16)
    ones_in = inp("ones16", (128, 128), F16)
    hmask_in = inp("hmask", (E, H), F16)
    cmask_in = inp("cmask", (H, E), F16)

    out_dram = nc.dram_tensor("out", [N, c.OUT], F32, kind="ExternalOutput")

    def idram(name):
        return [nc.dram_tensor(f"{name}_c{ci}", [E, C], F16).ap().rearrange(
            "(k p) n -> p k n", p=128) for ci in range(NC)]

    rs = {}
    for s in ("b", "l"):
        rs[s, 0] = idram(f"r{s}0")
        for l in range(L):
            for st in (1, 2, 3):
                rs[s, (l, st)] = idram(f"r{s}_{l}_{st}")

    lowp = nc.allow_low_precision("fp16 activations within rel-err budget")

    with tile.TileContext(nc) as tc, ExitStack() as ctx, lowp:
        p_ = ctx.enter_context
        cst = p_(tc.tile_pool(name="cst", bufs=1))
        wq = p_(tc.tile_pool(name="wq", bufs=2))       # big weights
        wcol = p_(tc.tile_pool(name="wcol", bufs=8))   # bias cols
        wrow = p_(tc.tile_pool(name="wrow", bufs=2))   # bias rows
        pa = p_(tc.tile_pool(name="pa", bufs=10))      # 4KB fp16 act tiles
        pb = p_(tc.tile_pool(name="pb", bufs=4))       # ffn h tiles
        pc = p_(tc.tile_pool(name="pc", bufs=9))       # small tiles
        pat = p_(tc.tile_pool(name="pat", bufs=2))     # attn persistents
        ps = p_(tc.tile_pool(name="ps", bufs=5, space="PSUM"))
        pskv = p_(tc.tile_pool(name="pskv", bufs=1, space="PSUM"))
        psst = p_(tc.tile_pool(name="psst", bufs=2, space="PSUM"))

        v, sc, gp = nc.vector, nc.scalar, nc.gpsimd

        def mm(out, lhsT, rhs, start, stop):
            nc.tensor.matmul(out, lhsT, rhs, start=start, stop=stop)

        # ---- constants ----
        ident_t = cst.tile([128, 128], F16, tag="ident")
        nc.sync.dma_start(out=ident_t, in_=ident_in)
        ones_t = cst.tile([128, 128], F16, tag="ones")
        nc.sync.dma_start(out=ones_t, in_=ones_in)
        hmask_t = cst.tile([128, KE, H], F16, tag="hmask")
        nc.sync.dma_start(out=hmask_t,
                          in_=hmask_in.rearrange("(k p) h -> p k h", p=128))
        cmask_t = cst.tile([H, KE, 128], F16, tag="cmask")
        nc.sync.dma_start(out=cmask_t,
                          in_=cmask_in.rearrange("h (k p) -> h k p", p=128))
        ONES_COL = ones_t[:, 0:1]
        ONES_ROW = ones_t[0:1, :]
        eps_den = cst.tile([8, 1], F32, tag="epsd")
        v.memset(eps_den, 1e-6 * DEN_SCALE)
        eps_ln = cst.tile([1, 1], F32, tag="epsl")
        v.memset(eps_ln, LN_EPS)

        def col_tile(src_ap, m, tag="col"):
            t = wcol.tile([128, m], F32, tag=tag)
            nc.sync.dma_start(out=t, in_=src_ap.rearrange("(m p) -> p m", p=128))
            return t

        def ln_pair(jobs, relu=False, apply_dve=False):
            """LayerNorm over features, pair-interleaved.
            jobs: list of (xs, g_col, b_col, outt); xs/outt fp16
            [128, KE, C]; outt doubles as x^2 scratch."""
            for xs, _, _, outt in jobs:
                v.tensor_tensor(out=outt, in0=xs, in1=xs, op=ALU.mult)
            pst = []
            for xs, _, _, outt in jobs:
                pp_s = psst.tile([8, C], F32, tag="st", name="lnst")
                pp_q = psst.tile([8, C], F32, tag="st", name="lnsq")
                for m in range(KE):
                    mm(pp_s[0:1, :], ONES_COL, xs[:, m, :], start=(m == 0),
                       stop=(m == KE - 1))
                for m in range(KE):
                    mm(pp_q[0:1, :], ONES_COL, outt[:, m, :], start=(m == 0),
                       stop=(m == KE - 1))
                pst.append((pp_s, pp_q))
            stts = []
            for (xs, _, _, _), (pp_s, pp_q) in zip(jobs, pst):
                arow = pc.tile([1, C], F32, tag="s2", bufs=4, name="arow")
                brow = pc.tile([1, C], F32, tag="s2", bufs=4, name="brow")
                sc.activation(arow, pp_s[0:1, :], AF.Copy, scale=1.0 / E)
                sc.activation(brow, pp_q[0:1, :], AF.Copy, scale=1.0 / E)
                trow2 = pc.tile([1, C], F32, tag="s2", bufs=4)
                sc.activation(trow2, arow, AF.Square)
                v.tensor_tensor(out=brow, in0=brow, in1=trow2,
                                op=ALU.subtract)
                sc.activation(brow, brow, AF.Sqrt, bias=eps_ln[0:1, 0:1])
                stt = pc.tile([1, 2, C], F16, tag="s2", bufs=4)
                v.reciprocal(out=stt[:, 0, :], in_=brow)
                v.tensor_tensor(out=stt[:, 1, :], in0=arow, in1=stt[:, 0, :],
                                op=ALU.mult)
                stts.append(stt)
            sbs = []
            for stt in stts:
                bc_s = ps.tile([128, C], F32, tag="mm")
                mm(bc_s, ones_t[0:1, :], stt[0:1, 0, :], start=True, stop=True)
                bc_t = ps.tile([128, C], F32, tag="mm")
                mm(bc_t, ones_t[0:1, :], stt[0:1, 1, :], start=True, stop=True)
                sb = pc.tile([128, 2, C], F16, tag="c2", bufs=2)
                sc.activation(sb[:, 0, :], bc_s, AF.Copy)
                sc.activation(sb[:, 1, :], bc_t, AF.Copy)
                sbs.append(sb)
            fn = AF.Relu if relu else AF.Identity
            for (xs, g_col, b_col, outt), sb in zip(jobs, sbs):
                sbb = sb[:, 0:1, :].broadcast_to([128, KE, C])
                v.tensor_tensor(out=xs, in0=xs, in1=sbb, op=ALU.mult)
                tbb = sb[:, 1:2, :].broadcast_to([128, KE, C])
                v.tensor_tensor(out=xs, in0=xs, in1=tbb, op=ALU.subtract)
            for m in range(KE):
                for (xs, g_col, b_col, outt), sb in zip(jobs, sbs):
                    if apply_dve:
                        v.tensor_scalar(out=outt[:, m, :], in0=xs[:, m, :],
                                        scalar1=g_col[:, m:m + 1],
                                        scalar2=b_col[:, m:m + 1],
                                        op0=ALU.mult, op1=ALU.add)
                        if relu:
                            v.tensor_scalar_max(outt[:, m, :], outt[:, m, :],
                                                0.0)
                    else:
                        sc.activation(outt[:, m, :], xs[:, m, :], fn,
                                      bias=b_col[:, m:m + 1],
                                      scale=g_col[:, m:m + 1])

        def load_x_chunk(dram_l1, ci, tag="a4"):
            xt = pa.tile([128, KE, C], F16, tag=tag)
            nc.sync.dma_start(out=xt, in_=dram_l1[ci])
            return xt

        def store_chunk(dram_l1, ci, t):
            gp.dma_start(out=dram_l1[ci], in_=t)

        # ---- entry transpose (interleaved) ----
        def entry_tile(x_ap, dst, ttk):
            x2 = pa.tile([128, E], F16, tag="a4")
            nc.sync.dma_start(out=x2, in_=x_ap[ttk * 128:(ttk + 1) * 128, :])
            xt = pa.tile([128, KE, 128], F16, tag="a4")
            for f in range(KE):
                pt = ps.tile([128, 128], F16, tag="mm")
                nc.tensor.transpose(pt, x2[:, f * 128:(f + 1) * 128], ident_t)
                if f % 2 == 0:
                    sc.activation(xt[:, f, :], pt, AF.Copy)
                else:
                    v.tensor_copy(xt[:, f, :], pt)
            tl = ttk % NTT
            nc.sync.dma_start(out=dst[ttk // NTT][:, :, tl * 128:(tl + 1) * 128],
                              in_=xt)

        PHASES.append(("entry", len(nc.inst_map)))
        for ttk in range(N // 128):
            entry_tile(body_feats, rs["b", 0], ttk)
            entry_tile(limb_feats, rs["l", 0], ttk)

        # ---- linear attention (pairs) ----
        def attn_setup(l, a):
            st = {}
            kvw = wq.tile([128, KE, 2, E], F16, tag="kvw")
            for t3 in (1, 2):
                nc.sync.dma_start(
                    out=kvw[:, :, t3 - 1, :],
                    in_=qkvw[l, a, t3].rearrange("(k p) e -> p k e", p=128))
            qw = wq.tile([128, KE, E], F16, tag="qw")
            nc.sync.dma_start(
                out=qw, in_=qkvw[l, a, 0].rearrange("(k p) e -> p k e", p=128))
            owt = wq.tile([128, KE, E], F16, tag="ow")
            nc.sync.dma_start(
                out=owt, in_=ow[l, a].rearrange("(k p) e -> p k e", p=128))
            st["kvw"], st["qw"], st["owt"] = kvw, qw, owt
            st["ubq_col"] = col_tile(ub[l, a, 0], KE)
            ubkv = wrow.tile([1, 2, E], F16, tag="row")
            nc.sync.dma_start(out=ubkv[:, 0, :], in_=ub16[l, a, 1][None, :])
            nc.sync.dma_start(out=ubkv[:, 1, :], in_=ub16[l, a, 2][None, :])
            st["ubkv"] = ubkv
            st["ob_col"] = col_tile(ob[l, a], KE)
            st["kv_acc"] = pat.tile([128, 4, 258], F32, tag="kva",
                                    name="kv_acc")
            return st

        def alpha_pair_step(sts, srcs, ci):
            xts = [load_x_chunk(src, ci) for src in srcs]
            work = []
            for st, xt in zip(sts, xts):
                k2f = pa.tile([128, NTT, E], F16, tag="a4", name="k2f")
                v2x = pa.tile([128, NTT, 2, 258], F16, tag="a4", name="v2x")
                v.memset(v2x[:, :, :, 256:258], 1.0)
                work.append((st, xt, k2f, v2x))
            for tt in range(NTT):
                for st, xt, k2f, v2x in work:
                    kvw = st["kvw"]
                    pk = ps.tile([128, E], F32, tag="mm")
                    pv = ps.tile([128, E], F32, tag="mm")
                    for k in range(KE):
                        lx = xt[:, k, tt * 128:(tt + 1) * 128]
                        mm(pk, lx, kvw[:, k, 0, :], start=(k == 0), stop=False)
                        mm(pv, lx, kvw[:, k, 1, :], start=(k == 0), stop=False)
                    mm(pk, ONES_ROW, st["ubkv"][:, 0, :], start=False,
                       stop=True)
                    mm(pv, ONES_ROW, st["ubkv"][:, 1, :], start=False,
                       stop=True)
                    ee = pc.tile([128, E], F16, tag="c1")
                    rr = pc.tile([128, E], F16, tag="c1")
                    sc.activation(ee, pk, AF.Exp)
                    v.tensor_scalar_max(rr, pk, 0.0)
                    v.tensor_scalar_min(ee, ee, 1.0)
                    v.tensor_tensor(out=k2f[:, tt, :], in0=ee, in1=rr,
                                    op=ALU.add)
                    sc.activation(v2x[:, tt, 0, 0:256], pv[:, 0:256], AF.Copy)
                    sc.activation(v2x[:, tt, 1, 0:256], pv[:, 256:512],
                                  AF.Copy)
            for p in range(4):
                for st, xt, k2f, v2x in work:
                    pkv = pskv.tile([128, 258], F32, tag="kv")
                    for tt in range(NTT):
                        mm(pkv, k2f[:, tt, p * 128:(p + 1) * 128],
                           v2x[:, tt, p // 2, :],
                           start=(tt == 0), stop=(tt == NTT - 1))
                    kv_acc = st["kv_acc"]
                    if ci == 0:
                        sc.activation(kv_acc[:, p, :], pkv, AF.Copy)
                    else:
                        v.tensor_tensor(out=kv_acc[:, p, :],
                                        in0=kv_acc[:, p, :], in1=pkv,
                                        op=ALU.add)

        def alpha_fin(st):
            kv_acc = st["kv_acc"]
            bd = pat.tile([128, KE, 128], F16, tag="bd")
            v.memset(bd, 0.0)
            for p in range(4):
                h0c = (2 * p % 4) * 64
                h1c = ((2 * p + 1) % 4) * 64
                v.tensor_scalar_mul(bd[0:64, p, 0:64],
                                    kv_acc[0:64, p, h0c:h0c + 64], DEN_SCALE)
                v.tensor_scalar_mul(bd[64:128, p, 64:128],
                                    kv_acc[64:128, p, h1c:h1c + 64], DEN_SCALE)
            kmm = pat.tile([128, KE, H], F16, tag="km")
            for k in range(KE):
                v.tensor_scalar(out=kmm[:, k, :], in0=hmask_t[:, k, :],
                                scalar1=kv_acc[:, k, 256:257],
                                scalar2=DEN_SCALE, op0=ALU.mult, op1=ALU.mult)
            st["bd"], st["kmm"] = bd, kmm

        def beta_pair_step(sts, srcs, ci, tails):
            xqs = [load_x_chunk(src, ci) for src in srcs]
            qfs = [pa.tile([128, KE, C], F16, tag="a4", name="qf")
                   for _ in sts]
            pds = [psst.tile([8, C], F32, tag="st", name="pd")
                   for _ in sts]
            for m in range(KE):
                for st, xq, qf, pd in zip(sts, xqs, qfs, pds):
                    qw = st["qw"]
                    pq = ps.tile([128, C], F32, tag="mm")
                    for k in range(KE):
                        mm(pq, qw[:, k, m * 128:(m + 1) * 128],
                           xq[:, k, :], start=(k == 0), stop=(k == KE - 1))
                    ee = pc.tile([128, C], F16, tag="c1")
                    rr = pc.tile([128, C], F16, tag="c1")
                    sc.activation(ee, pq, AF.Exp,
                                  bias=st["ubq_col"][:, m:m + 1])
                    v.tensor_scalar(out=rr, in0=pq,
                                    scalar1=st["ubq_col"][:, m:m + 1],
                                    scalar2=0.0, op0=ALU.add, op1=ALU.max)
                    v.tensor_scalar_min(ee, ee, 1.0)
                    v.tensor_tensor(out=qf[:, m, :], in0=ee, in1=rr,
                                    op=ALU.add)
                    mm(pd, st["kmm"][:, m, :], qf[:, m, :], start=(m == 0),
                       stop=(m == KE - 1))
            recs = []
            for st, pd in zip(sts, pds):
                rec = pc.tile([8, C], F16, tag="s2", bufs=4)
                v.reciprocal(out=rec, in_=pd)
                recs.append(rec)
            atts = [pa.tile([128, KE, C], F16, tag="a4", name="att")
                    for _ in sts]
            for m in range(KE):
                for st, qf, att, rec in zip(sts, qfs, atts, recs):
                    pn = ps.tile([128, C], F32, tag="mm")
                    mm(pn, st["bd"][:, m, :], qf[:, m, :], start=True,
                       stop=True)
                    pr = ps.tile([128, C], F32, tag="mm")
                    mm(pr, cmask_t[:, m, :], rec, start=True, stop=True)
                    rb = pc.tile([128, C], F16, tag="c1")
                    sc.activation(rb, pr, AF.Copy)
                    v.tensor_tensor(out=att[:, m, :], in0=pn, in1=rb,
                                    op=ALU.mult)
            projs = [pa.tile([128, KE, C], F16, tag="a4", name="proj")
                     for _ in sts]
            for m in range(KE):
                for st, att, proj in zip(sts, atts, projs):
                    po = ps.tile([128, C], F32, tag="mm")
                    for k in range(KE):
                        mm(po, st["owt"][:, k, m * 128:(m + 1) * 128],
                           att[:, k, :], start=(k == 0), stop=(k == KE - 1))
                    sc.activation(proj[:, m, :], po, AF.Identity,
                                  bias=st["ob_col"][:, m:m + 1])
            tails[0](ci, projs, xqs)

        # ---- tails (pair) ----
        def make_self_tail_pair(l, dsts):
            cols = []
            for i, s in enumerate(("b", "l")):
                g_col = col_tile(lng[l, i], KE, tag="lncol")
                b_col = col_tile(lnb[l, i], KE, tag="lncol")
                cols.append((g_col, b_col))

            def tail(ci, projs, xqs):
                jobs = []
                for (g_col, b_col), proj, xq, dst in zip(cols, projs, xqs,
                                                         dsts):
                    v.tensor_tensor(out=proj, in0=proj, in1=xq, op=ALU.add)
                for (g_col, b_col), proj, xq, dst in zip(cols, projs, xqs,
                                                         dsts):
                    outt = pa.tile([128, KE, C], F16, tag="a4", name="outt")
                    jobs.append((proj, g_col, b_col, outt))
                ln_pair(jobs)
                for (j, dst) in zip(jobs, dsts):
                    store_chunk(dst, ci, j[3])

            return tail

        def make_cross_tail_pair(l, dsts):
            gw1t = wq.tile([128, 2 * KE, E4], F16, tag="gw")
            nc.sync.dma_start(out=gw1t,
                              in_=gw1[l].rearrange("(k p) g -> p k g", p=128))
            gwd_col = wcol.tile([128, 1], F16, tag="gwd")
            nc.sync.dma_start(out=gwd_col,
                              in_=gwd[l].rearrange("(m p) -> p m", p=128))
            gb1_col = col_tile(gb1[l], 1, tag="lncol")
            gb2d_t = wcol.tile([1, 1], F32, tag="gb2d")
            nc.sync.dma_start(out=gb2d_t, in_=gb2d[l][None, :])
            g_col = col_tile(lng[l, 2], KE, tag="lncol")
            b_col = col_tile(lnb[l, 2], KE, tag="lncol")

            def tail(ci, projs, xqs):
                bgts = []
                for proj, xq in zip(projs, xqs):
                    pg = ps.tile([128, C], F32, tag="mm")
                    for k in range(2 * KE):
                        rhs = xq[:, k, :] if k < KE else proj[:, k - KE, :]
                        mm(pg, gw1t[:, k, :], rhs, start=(k == 0),
                           stop=(k == 2 * KE - 1))
                    g1f = pc.tile([128, C], F16, tag="c1")
                    sc.activation(g1f, pg, AF.Relu, bias=gb1_col[:, 0:1])
                    g1t = pc.tile([128, C], F16, tag="c1")
                    v.tensor_scalar_min(g1t, g1f, 6.0)
                    pg2 = psst.tile([8, C], F32, tag="st", name="pg2")
                    mm(pg2[0:1, :], gwd_col, g1t, start=True, stop=True)
                    bgf = pc.tile([1, C], F16, tag="s2", bufs=4)
                    sc.activation(bgf, pg2[0:1, :], AF.Sigmoid,
                                  bias=gb2d_t[0:1, 0:1])
                    pbg = ps.tile([128, C], F32, tag="mm")
                    mm(pbg, ones_t[0:1, :], bgf, start=True, stop=True)
                    bgt = pc.tile([128, C], F16, tag="c1")
                    sc.activation(bgt, pbg, AF.Copy)
                    bgts.append(bgt)
                jobs = []
                for proj, xq, bgt in zip(projs, xqs, bgts):
                    mt = pa.tile([128, KE, C], F16, tag="a4", name="mt")
                    v.tensor_tensor(out=mt, in0=xq, in1=proj, op=ALU.subtract)
                    bgb = bgt.unsqueeze(1).broadcast_to([128, KE, C])
                    v.tensor_tensor(out=mt, in0=mt, in1=bgb, op=ALU.mult)
                    v.tensor_tensor(out=mt, in0=mt, in1=proj, op=ALU.add)
                    outt = pa.tile([128, KE, C], F16, tag="a4", name="outt")
                    jobs.append((mt, g_col, b_col, outt))
                ln_pair(jobs)
                for (j, dst) in zip(jobs, dsts):
                    store_chunk(dst, ci, j[3])

            return tail

        # ---- FFN pair ----
        def ffn_setup(l, s):
            si = 0 if s == "b" else 1
            st = {}
            w1t = wq.tile([128, KE, X], F16, tag="w1")
            nc.sync.dma_start(
                out=w1t, in_=w1[l, si].rearrange("(k p) x -> p k x", p=128))
            w2t = wq.tile([128, KX, E], F16, tag="w2")
            nc.sync.dma_start(
                out=w2t, in_=w2[l, si].rearrange("(k p) e -> p k e", p=128))
            tapt = wq.tile([128, KX, 3, 128], F16, tag="tp" + s, bufs=1)
            nc.sync.dma_start(out=tapt,
                              in_=taps[l, si].rearrange("m t p f -> p m t f"))
            st["w1t"], st["w2t"], st["tapt"] = w1t, w2t, tapt
            st["b1_col"] = col_tile(b1[l, si], KX, tag="ffcol")
            st["b2_col"] = col_tile(b2[l, si], KE, tag="ffcol")
            st["B_col"] = col_tile(convB[l, si], KX, tag="ffcol")
            st["g_col"] = col_tile(lng[l, 3 if s == "b" else 4], KE,
                                   tag="lncol")
            st["bb_col"] = col_tile(lnb[l, 3 if s == "b" else 4], KE,
                                    tag="lncol")
            st["hts"] = [None] * NC
            st["xts"] = [None] * NC
            return st

        def ffn_h_pair(sts, srcs, ci):
            for st, src in zip(sts, srcs):
                xt = load_x_chunk(src, ci)
                st["xts"][ci] = xt
                ht = pb.tile([128, KX, C + 2], F16, tag="ht")
                if ci == 0:
                    v.memset(ht[:, :, 0:1], 0.0)
                st["hts"][ci] = ht
            for m in range(KX):
                for st in sts:
                    ht, xt = st["hts"][ci], st["xts"][ci]
                    ph = ps.tile([128, C], F32, tag="mm")
                    for k in range(KE):
                        mm(ph, st["w1t"][:, k, m * 128:(m + 1) * 128],
                           xt[:, k, :], start=(k == 0), stop=(k == KE - 1))
                    sc.activation(ht[:, m, 1:C + 1], ph, AF.Relu,
                                  bias=st["b1_col"][:, m:m + 1])
                    v.tensor_scalar_min(ht[:, m, 1:C + 1], ht[:, m, 1:C + 1],
                                        6.0)
            for st in sts:
                ht = st["hts"][ci]
                prev = st["hts"][ci - 1] if ci > 0 else None
                if prev is not None:
                    v.tensor_copy(prev[:, :, C + 1:C + 2], ht[:, :, 1:2])
                    v.tensor_copy(ht[:, :, 0:1], prev[:, :, C:C + 1])
                if ci == NC - 1:
                    v.memset(ht[:, :, C + 1:C + 2], 0.0)

        def ffn_tail_pair(sts, dsts, ci):
            h2s = []
            for st in sts:
                h2 = pb.tile([128, KX, C], F16, tag="h2", bufs=2, name="h2")
                h2s.append(h2)
            for m in range(KX):
                for st, h2 in zip(sts, h2s):
                    ht = st["hts"][ci]
                    pacc = ps.tile([128, C], F32, tag="mm")
                    for t in range(3):
                        mm(pacc, st["tapt"][:, m, t, :], ht[:, m, t:t + C],
                           start=(t == 0), stop=(t == 2))
                    rel = pc.tile([128, C], F16, tag="c1")
                    sc.activation(rel, pacc, AF.Relu,
                                  bias=st["B_col"][:, m:m + 1])
                    v.tensor_scalar_min(h2[:, m, :], rel, 6.0)
            rts = []
            for st, h2 in zip(sts, h2s):
                rt = pa.tile([128, KE, C], F16, tag="a4", name="rt")
                rts.append(rt)
            for m in range(KE):
                for st, h2, rt in zip(sts, h2s, rts):
                    pw = ps.tile([128, C], F32, tag="mm")
                    for k in range(KX):
                        mm(pw, st["w2t"][:, k, m * 128:(m + 1) * 128],
                           h2[:, k, :], start=(k == 0), stop=(k == KX - 1))
                    sc.activation(rt[:, m, :], pw, AF.Identity,
                                  bias=st["b2_col"][:, m:m + 1])
            jobs = []
            for st, rt, dst in zip(sts, rts, dsts):
                v.tensor_tensor(out=rt, in0=rt, in1=st["xts"][ci], op=ALU.add)
                outt = pa.tile([128, KE, C], F16, tag="a4", name="outt")
                jobs.append((rt, st["g_col"], st["bb_col"], outt))
            ln_pair(jobs)
            for j, dst in zip(jobs, dsts):
                store_chunk(dst, ci, j[3])
            for st in sts:
                st["hts"][ci] = st["xts"][ci] = None

        # ---- layers ----
        for l in range(L):
            bsrc = rs["b", 0] if l == 0 else rs["b", (l - 1, 3)]
            lsrc = rs["l", 0] if l == 0 else rs["l", (l - 1, 3)]

            PHASES.append((f"attnA{l}.alpha", len(nc.inst_map)))
            stA = [attn_setup(l, 0), attn_setup(l, 1)]
            for ci in range(NC):
                alpha_pair_step(stA, [bsrc, lsrc], ci)
            for st in stA:
                alpha_fin(st)
            PHASES.append((f"attnA{l}.beta", len(nc.inst_map)))
            tailA = make_self_tail_pair(l, [rs["b", (l, 1)], rs["l", (l, 1)]])
            for ci in range(NC):
                beta_pair_step(stA, [bsrc, lsrc], ci, [tailA])

            PHASES.append((f"attnB{l}.alpha", len(nc.inst_map)))
            b1d, l1d = rs["b", (l, 1)], rs["l", (l, 1)]
            stB = [attn_setup(l, 2), attn_setup(l, 3)]
            for ci in range(NC):
                alpha_pair_step(stB, [l1d, b1d], ci)
            for st in stB:
                alpha_fin(st)
            PHASES.append((f"attnB{l}.beta", len(nc.inst_map)))
            tailB = make_cross_tail_pair(l, [rs["b", (l, 2)],
                                             rs["l", (l, 2)]])
            for ci in range(NC):
                beta_pair_step(stB, [b1d, l1d], ci, [tailB])

            PHASES.append((f"ffn{l}", len(nc.inst_map)))
            stF = [ffn_setup(l, "b"), ffn_setup(l, "l")]
            fsrc = [rs["b", (l, 2)], rs["l", (l, 2)]]
            fdst = [rs["b", (l, 3)], rs["l", (l, 3)]]
            ffn_h_pair(stF, fsrc, 0)
            for ci in range(1, NC):
                ffn_h_pair(stF, fsrc, ci)
                ffn_tail_pair(stF, fdst, ci - 1)
            ffn_tail_pair(stF, fdst, NC - 1)

        PHASES.append(("final", len(nc.inst_map)))
        # ---- final head ----
        fw1t = wq.tile([128, 2 * KE, E2], F16, tag="w1")
        nc.sync.dma_start(out=fw1t,
                          in_=fw1.rearrange("(k p) g -> p k g", p=128))
        fw2t = wq.tile([128, 2, E], F16, tag="gw")
        nc.sync.dma_start(out=fw2t,
                          in_=fw2.rearrange("(k p) e -> p k e", p=128))
        rw1t = wq.tile([128, KE, E4], F16, tag="gw")
        nc.sync.dma_start(out=rw1t,
                          in_=rw1.rearrange("(k p) g -> p k g", p=128))
        rw2t = wrow.tile([128, E8], F16, tag="row2")
        nc.sync.dma_start(out=rw2t, in_=rw2)
        rw3t = wrow.tile([E8, 16], F16, tag="row2")
        nc.sync.dma_start(out=rw3t, in_=rw3p)
        rb3_row = wrow.tile([1, 16], F16, tag="row")
        nc.sync.dma_start(out=rb3_row, in_=rb3p)
        fb1_col = col_tile(fb1, 2, tag="fcol")
        fb2_col = col_tile(fb2, KE, tag="fcol")
        flng_col = col_tile(flng, KE, tag="fcol")
        flnb_col = col_tile(flnb, KE, tag="fcol")
        rb1_col = col_tile(rb1, 1, tag="fcol")
        rb2_col = wcol.tile([E8, 1], F32, tag="fcol")
        nc.sync.dma_start(out=rb2_col, in_=rb2[:, None])
        out_ap = out_dram.ap()

        bsrc, lsrc = rs["b", (L - 1, 3)], rs["l", (L - 1, 3)]
        for cp in range(0, NC, 2):
            prs = []
            for ci in (cp, cp + 1):
                xb = load_x_chunk(bsrc, ci)
                xl = load_x_chunk(lsrc, ci)
                f1t = [pc.tile([128, C], F16, tag="c1", name=f"f1t{_i}")
                       for _i in range(2)]
                prs.append({"ci": ci, "xb": xb, "xl": xl, "f1t": f1t})
            for m in range(2):
                for pr_ in prs:
                    pf = ps.tile([128, C], F32, tag="mm")
                    for k in range(2 * KE):
                        rhs = (pr_["xb"][:, k, :] if k < KE
                               else pr_["xl"][:, k - KE, :])
                        mm(pf, fw1t[:, k, m * 128:(m + 1) * 128], rhs,
                           start=(k == 0), stop=(k == 2 * KE - 1))
                    f1f = pc.tile([128, C], F16, tag="c1")
                    v.tensor_scalar(out=f1f, in0=pf,
                                    scalar1=fb1_col[:, m:m + 1],
                                    scalar2=0.0, op0=ALU.add, op1=ALU.max)
                    v.tensor_scalar_min(pr_["f1t"][m], f1f, 6.0)
            for pr_ in prs:
                pr_["ft"] = pa.tile([128, KE, C], F16, tag="a4", name="ft")
            for m in range(KE):
                for pr_ in prs:
                    pf2 = ps.tile([128, C], F32, tag="mm")
                    for k in range(2):
                        mm(pf2, fw2t[:, k, m * 128:(m + 1) * 128],
                           pr_["f1t"][k], start=(k == 0), stop=(k == 1))
                    sc.activation(pr_["ft"][:, m, :], pf2, AF.Identity,
                                  bias=fb2_col[:, m:m + 1])
            jobs = []
            for pr_ in prs:
                frt = pa.tile([128, KE, C], F16, tag="a4", name="frt")
                pr_["frt"] = frt
                jobs.append((pr_["ft"], flng_col, flnb_col, frt))
            ln_pair(jobs, relu=True)
            for pr_ in prs:
                p1 = ps.tile([128, C], F32, tag="mm")
                for k in range(KE):
                    mm(p1, rw1t[:, k, :], pr_["frt"][:, k, :],
                       start=(k == 0), stop=(k == KE - 1))
                pr_["p1"] = p1
            for pr_ in prs:
                h1f = pc.tile([128, C], F16, tag="c1")
                v.tensor_scalar(out=h1f, in0=pr_["p1"],
                                scalar1=rb1_col[:, 0:1],
                                scalar2=0.0, op0=ALU.add, op1=ALU.max)
                h1t = pc.tile([128, C], F16, tag="c1")
                v.tensor_scalar_min(h1t, h1f, 6.0)
                pr_["p2"] = ps.tile([E8, C], F32, tag="mm", name="p2")
                mm(pr_["p2"], rw2t, h1t, start=True, stop=True)
            for pr_ in prs:
                h2f = pc.tile([E8, C], F16, tag="c1")
                sc.activation(h2f, pr_["p2"], AF.Relu, bias=rb2_col[:, 0:1])
                h2t = pc.tile([E8, C], F16, tag="c1")
                v.tensor_scalar_min(h2t, h2f, 6.0)
                pr_["h2t"] = h2t
            for pr_ in prs:
                ot = pc.tile([128, NTT, c.OUT], F32, tag="c2", bufs=2)
                for tt in range(NTT):
                    p3 = ps.tile([128, 16], F32, tag="mm")
                    mm(p3, pr_["h2t"][:, tt * 128:(tt + 1) * 128], rw3t,
                       start=True, stop=False)
                    mm(p3, ONES_ROW[:, 0:128], rb3_row, start=False, stop=True)
                    sc.activation(ot[:, tt, :], p3[:, 0:c.OUT], AF.Copy)
                ci = pr_["ci"]
                nc.sync.dma_start(
                    out=out_ap[ci * C:(ci + 1) * C, :].rearrange(
                        "(tt p) o -> p tt o", p=128),
                    in_=ot)

    return din, out_dram


# ======================================================================
# kernel() entry point: full inputs in, full outputs out (8-core SPMD).
# ======================================================================
import concourse.bacc as _bacc
from concourse.bass_utils import run_bass_kernel_spmd as _run_spmd

_N_CORES = 8
_CACHE = {}


def _get_nc():
    if "nc" not in _CACHE:
        nc = _bacc.Bacc("TRN2", target_bir_lowering=False, debug=False)
        build(nc, Cfg())
        nc.finalize()
        _CACHE["nc"] = nc
    return _CACHE["nc"]


def kernel(**inputs):
    nc = _get_nc()
    cfg = Cfg()
    arr = {k: np.asarray(v) for k, v in inputs.items()}
    consts = host_constants(cfg, arr)
    shared = {k: a for k, a in consts.items()
              if k not in ("body_feats", "limb_feats")}
    in_maps = []
    for i in range(_N_CORES):
        m = dict(shared)
        m["body_feats"] = np.ascontiguousarray(consts["body_feats"][i])
        m["limb_feats"] = np.ascontiguousarray(consts["limb_feats"][i])
        in_maps.append(m)
    res = run_kernel_spmd_cached(nc, in_maps)
    out = np.stack([res[i]["out"] for i in range(_N_CORES)], axis=0)
    return out.astype(np.float32)


def run_kernel_spmd_cached(nc, in_maps, **kw):
    r = _run_spmd(nc, in_maps, list(range(_N_CORES)), **kw)
    _CACHE["last_result"] = r
    return r.results
